# revision 1
# baseline (speedup 1.0000x reference)
"""Trainium2 Bass kernel for nn_CMFuser (topk_masking).

Self-contained: accepts FULL inputs (as produced by setup_inputs()), returns
the FULL [32, 512, 768] output. Internally shards batch across 8 NeuronCores
(pure data parallel, 4 batches/core) and runs a hand-written Bass/Tile kernel.

Algorithmic structure (validated against the jax reference to ~1e-3):
  * BN(eval) + topk-channel-exchange blend folds into per-channel affine:
        x0_rgb = A1*rgb + A2*depth + A3,   x0_depth = D1*depth + D2*rgb + D3
  * The 2-token attention with -1e9 diag mask is an EXACT token swap
    (exp(-1e9) underflows to 0 in f32), so qkv+softmax+proj collapse into
    one fused C x C matmul Wc = proj_w @ Wv applied to the OTHER token.
  * LN weights fold into the following matmul; LN mean-subtraction folds
    into a rank-1 (K=1) matmul correction on the output.
  * Final LN + mean over the 2 modality tokens folds into 0.5*wf scale.

Device layout: channel-major activations [128 channels, 512 tokens] per tile;
LN statistics via ones-matmuls on the PE; per-token broadcast via K=1 matmuls.
Matmuls for Wc/fc1/fc2 run in bf16 (weights + normalized activations);
everything else (residual stream, statistics) stays f32 / f32r.
"""

import os
import sys

sys.path.insert(0, "/opt/trn_rl_repo")

import numpy as np
import ml_dtypes

import concourse.bass as bass
import concourse.mybir as mybir
import concourse.tile as tile
from concourse.bass_utils import run_bass_kernel_spmd
from contextlib import ExitStack

dt = mybir.dt
Alu = mybir.AluOpType
Act = mybir.ActivationFunctionType

B, T, C = 32, 512, 768
H = 4
K_EX = int(C * 0.2)
MLP = 4 * C
EPS = 1e-5
N_CORES = 8
B_CORE = B // N_CORES          # 4 batches per core
ROWS = B_CORE * T              # 2048 token-sites per core
TG = 512                       # tokens per group (= T)
NG = ROWS // TG                # groups per core
CT = C // 128                  # 6 channel tiles
MT = MLP // 128                # 24 mlp tiles
NTT = TG // 128                # 4 token tiles per group

# vector slot indices in the packed per-channel constant table
V_A1, V_A2, V_A3, V_D1, V_D2, V_D3, V_PB, V_FC2B, V_WFH, V_BF = range(10)
NV = 10

_CACHE = {}


def _build_nc(act_fn=None, n_groups=NG, legalize=True):
    """Build the per-core Bass module (same program on all 8 cores)."""
    if act_fn is None:
        act_fn = Act.Gelu
    nc = bass.Bass()

    rgb_d = nc.dram_tensor("rgb", [ROWS, C], dt.float32, kind="ExternalInput")
    dep_d = nc.dram_tensor("dep", [ROWS, C], dt.float32, kind="ExternalInput")
    wc_d = nc.dram_tensor("wc", [128, CT * C], dt.bfloat16, kind="ExternalInput")
    fc1_d = nc.dram_tensor("fc1", [128, CT * MLP], dt.bfloat16, kind="ExternalInput")
    fc2_d = nc.dram_tensor("fc2", [128, MT * C], dt.bfloat16, kind="ExternalInput")
    vecs_d = nc.dram_tensor("vecs", [128, CT * NV], dt.float32, kind="ExternalInput")
    fb1_d = nc.dram_tensor("fb1", [128, MT], dt.float32, kind="ExternalInput")
    wcsum_d = nc.dram_tensor("wcsum", [1, C], dt.bfloat16, kind="ExternalInput")
    fc1sum_d = nc.dram_tensor("fc1sum", [1, MLP], dt.bfloat16, kind="ExternalInput")
    ident_d = nc.dram_tensor("ident", [128, 128], dt.float32, kind="ExternalInput")
    out_d = nc.dram_tensor("out", [ROWS, C], dt.float32, kind="ExternalOutput")

    f32r = dt.float32r

    with tile.TileContext(nc) as tc, ExitStack() as ctx:
        const = ctx.enter_context(tc.tile_pool(name="const", bufs=1))
        inp = ctx.enter_context(tc.tile_pool(name="inp", bufs=8))
        resp = ctx.enter_context(tc.tile_pool(name="resp", bufs=14 if TG == 512 else 26))
        hp = ctx.enter_context(tc.tile_pool(name="hp", bufs=12 if TG == 512 else 24))
        sqp = ctx.enter_context(tc.tile_pool(name="sqp", bufs=2 if TG == 512 else 4))
        xbp = ctx.enter_context(tc.tile_pool(name="xbp", bufs=3 if TG == 512 else 5))
        tmpp = ctx.enter_context(tc.tile_pool(name="tmpp", bufs=3 if TG == 512 else 4))
        apool = ctx.enter_context(tc.tile_pool(name="apool", bufs=2 if TG == 512 else 3))
        rows2 = ctx.enter_context(tc.tile_pool(name="rows2", bufs=4))
        rows1 = ctx.enter_context(tc.tile_pool(name="rows1", bufs=4))
        uaffp = ctx.enter_context(tc.tile_pool(name="uaffp", bufs=6))
        outp = ctx.enter_context(tc.tile_pool(name="outp", bufs=1 if TG == 512 else 2))
        psum = ctx.enter_context(
            tc.tile_pool(name="psum", bufs=2, space="PSUM")
        )

        # ---- constants / weights (small, early-needed tensors first) ----
        ident_sb = const.tile([128, 128], dt.float32)
        nc.sync.dma_start(ident_sb[:], ident_d[:])
        vecs_sb = const.tile([128, CT * NV], dt.float32)
        nc.sync.dma_start(vecs_sb[:], vecs_d[:])
        fb1_sb = const.tile([128, MT], dt.float32)
        nc.sync.dma_start(fb1_sb[:], fb1_d[:])
        wcsum_sb = const.tile([1, C], dt.bfloat16)
        nc.sync.dma_start(wcsum_sb[:], wcsum_d[:])
        fc1sum_sb = const.tile([1, MLP], dt.bfloat16)
        nc.sync.dma_start(fc1sum_sb[:], fc1sum_d[:])
        # group-0 inputs issued BEFORE the big weight loads so the first
        # transposes are not stuck behind ~14MB of weight DMA in the queues
        pre_in = {}
        for s_, src_ in ((0, rgb_d), (1, dep_d)):
            for tt_ in range(NTT):
                it_ = inp.tile([128, C], dt.float32, tag="in",
                               name=f"in_0_{s_}_{tt_}")
                nc.sync.dma_start(
                    it_[:], src_[tt_ * 128 : (tt_ + 1) * 128, :])
                nc.scalar.copy(it_[:], it_[:])
                pre_in[s_, tt_] = it_
        wc_sb = const.tile([128, CT * C], dt.bfloat16)
        nc.sync.dma_start(wc_sb[:], wc_d[:])
        fc1_sb = const.tile([128, CT * MLP], dt.bfloat16)
        nc.sync.dma_start(fc1_sb[:], fc1_d[:])
        fc2_sb = const.tile([128, MT * C], dt.bfloat16)
        nc.sync.dma_start(fc2_sb[:], fc2_d[:])
        # PE matmuls can carry only one sync wait; HWDGE DMAs may split
        # across queues (multiple semaphores). Interpose a no-op compute
        # touch on every DMA-produced tensor the PE reads directly.
        for _t in (ident_sb, vecs_sb, fb1_sb, wcsum_sb, fc1sum_sb):
            nc.scalar.copy(_t[:], _t[:])
        # Each ISA instruction carries at most ONE sync wait. Make the other
        # engines observe the ACT guard-copy clock once, up front, so later
        # per-instruction waits collapse to a single new semaphore.
        obs = const.tile([1, 4], dt.float32)
        nc.vector.tensor_copy(obs[0:1, 0:1], vecs_sb[0:1, 0:1])
        nc.gpsimd.memset(obs[0:1, 2:3], 0.0)

        ones_col = const.tile([128, 1], dt.bfloat16)
        nc.vector.memset(ones_col[:], 1.0)
        # bcast lhsT rows with folded sqrt(C) scaling (see ln_stats)
        sqrtc_f32 = const.tile([1, 128], dt.float32)
        nc.vector.memset(sqrtc_f32[:], float(np.sqrt(C)))
        sqrtc_row = const.tile([1, 128], dt.float32r)
        with nc.allow_low_precision("fp32r bcast lhsT"):
            nc.vector.tensor_copy(sqrtc_row[:], sqrtc_f32[:])
        isqrtc_row_b = const.tile([1, 128], dt.bfloat16)
        nc.vector.memset(isqrtc_row_b[:], float(1.0 / np.sqrt(C)))
        ceps_ap = const.tile([1, 1], dt.float32)
        nc.vector.memset(ceps_ap[:], float(C * EPS))

        def vec(idx, j):
            # per-channel scalar [128,1] for channel tile j
            return vecs_sb[:, j * NV + idx : j * NV + idx + 1]

        def ln_stats(xr, xd, name):
            """LN stats over the channel dim, per stream.

            xr/xd: lists of 6 [128,512] f32 SBUF tiles (channel-major).
            Returns dict with [1,512] SBUF rows: r_r, r_d (rsqrt) and
            mr_r, mr_d (mean*rsqrt).
            """
            out = {}
            for s, tiles in ((0, xr), (1, xd)):
                sfx = "r" if s == 0 else "d"
                xb = []
                sq = []
                for j in range(CT):
                    xbt = xbp.tile([128, TG], dt.bfloat16, tag="xb",
                                   name=f"xb_{name}_{s}_{j}")
                    nc.gpsimd.tensor_copy(xbt[:], tiles[j][:])
                    xb.append(xbt)
                    sqt = sqp.tile([128, TG], dt.bfloat16, tag="sq",
                                   name=f"sq_{name}_{s}_{j}")
                    nc.scalar.square(sqt[:], xbt[:])
                    sq.append(sqt)
                stat = psum.tile([128, TG], dt.float32, tag="acc", bufs=6,
                                 name=f"stat_{name}_{s}")
                # sum(x) accumulates at partition 0 (col-group 0) while
                # sum(x^2) accumulates at partition 32 (col-group 1); the
                # two M=1 matmul chains share the PE via col tiling.
                for j in range(CT):
                    nc.tensor.matmul(stat[0:1, :], ones_col[:],
                                     xb[j][:], tile_position=(0, 0),
                                     start=(j == 0), stop=(j == CT - 1))
                    nc.tensor.matmul(stat[32:33, :], ones_col[:],
                                     sq[j][:], tile_position=(0, 32),
                                     start=(j == 0), stop=(j == CT - 1))
                # With S1 = sum(x), S2 = sum(x^2):
                #   rr  = 1/sqrt(S2 - S1^2/C + C*eps) = rsqrt(var+eps)/sqrt(C)
                #   bcast of r uses a sqrt(C)-valued lhsT row;
                #   mr' = S1*rr = m*r*sqrt(C); the 1/sqrt(C) is folded into
                #   the wcsum/fc1sum correction rows host-side.
                sq1 = rows2.tile([1, TG], dt.float32, tag="rows",
                                 name=f"sq1_{name}_{s}")
                nc.scalar.square(sq1[:], stat[0:1, :])
                u = rows2.tile([1, TG], dt.float32, tag="rows",
                               name=f"u_{name}_{s}")
                nc.vector.scalar_tensor_tensor(u[:], sq1[:], -1.0 / C,
                                               stat[32:33, :], Alu.mult, Alu.add)
                std = rows2.tile([1, TG], dt.float32, tag="rows",
                                 name=f"std_{name}_{s}")
                nc.scalar.activation(std[:], u[:], Act.Sqrt,
                                     bias=ceps_ap[0:1, 0:1], scale=1.0)
                rrow = rows1.tile([1, TG], dt.float32r, tag="rows1", bufs=4,
                                  name=f"r_{name}_{s}")
                with nc.allow_low_precision("fp32r bcast rows"):
                    nc.vector.reciprocal(rrow[:], std[:])
                mr = rows1.tile([1, TG], dt.bfloat16, tag="rows1b", bufs=4,
                                name=f"mr_{name}_{s}")
                nc.vector.tensor_tensor(mr[:], stat[0:1, :], rrow[:], Alu.mult)
                out[f"r_{sfx}"] = rrow[:]
                out[f"mr_{sfx}"] = mr[:]
            return out

        def bcast(row_ap, name, tag="ps"):
            """Broadcast a [1,512] SBUF row across 128 partitions via K=1 MM."""
            bc = psum.tile([128, TG], dt.float32, tag=tag, name=f"bc_{name}")
            nc.tensor.matmul(bc[:], sqrtc_row[0:1, :],
                             row_ap, start=True, stop=True)
            return bc

        # ================= main loop over groups =================
        for g in range(n_groups):
            r0 = g * TG
            # ---- stage L: load token-major, PE-transpose, blend ----
            if g == 0:
                in_tiles = pre_in
            else:
                in_tiles = {}
                for s, src in ((0, rgb_d), (1, dep_d)):
                    for tt in range(NTT):
                        it = inp.tile([128, C], dt.float32, tag="in",
                                      name=f"in_{g}_{s}_{tt}")
                        nc.sync.dma_start(
                            it[:], src[r0 + tt * 128 : r0 + (tt + 1) * 128, :])
                        nc.scalar.copy(it[:], it[:])
                        in_tiles[s, tt] = it
            x = {}          # (s, j) -> [128, TG] f32 residual tiles
            for j in range(CT):
                pt = {}
                for s in (0, 1):
                    p = psum.tile([128, TG], dt.float32, tag="acc", bufs=6,
                                  name=f"pt_{g}_{s}_{j}")
                    for tt in range(NTT):
                        nc.tensor.transpose(
                            p[:, tt * 128 : (tt + 1) * 128],
                            in_tiles[s, tt][:, j * 128 : (j + 1) * 128],
                            ident_sb[:])
                    pt[s] = p
                t1 = tmpp.tile([128, TG], dt.float32, tag="bl",
                               name=f"t1_{g}_{j}")
                nc.vector.tensor_scalar(t1[:], pt[1][:], vec(V_A2, j),
                                        vec(V_A3, j), Alu.mult, Alu.add)
                x0r = resp.tile([128, TG], dt.float32, tag="res",
                                name=f"x0r_{g}_{j}")
                nc.vector.scalar_tensor_tensor(x0r[:], pt[0][:], vec(V_A1, j),
                                               t1[:], Alu.mult, Alu.add)
                t2 = tmpp.tile([128, TG], dt.float32, tag="bl",
                               name=f"t2_{g}_{j}")
                nc.vector.tensor_scalar(t2[:], pt[0][:], vec(V_D2, j),
                                        vec(V_D3, j), Alu.mult, Alu.add)
                x0d = resp.tile([128, TG], dt.float32, tag="res",
                                name=f"x0d_{g}_{j}")
                nc.vector.scalar_tensor_tensor(x0d[:], pt[1][:], vec(V_D1, j),
                                               t2[:], Alu.mult, Alu.add)
                x[0, j] = x0r
                x[1, j] = x0d

            # ---- norm1 + attention (exact swap) ----
            st1 = ln_stats([x[0, j] for j in range(CT)],
                           [x[1, j] for j in range(CT)], f"n1_{g}")
            if g == 0:
                # big-weight DMA guards on the idle GPSIMD, issued after the
                # norm1 stat copies so they block neither ACT squares nor the
                # early xb casts (ACT guards here stalled the PE ~27us)
                nc.gpsimd.tensor_copy(wc_sb[:], wc_sb[:])
                nc.gpsimd.tensor_copy(fc1_sb[:], fc1_sb[:])
                nc.gpsimd.tensor_copy(fc2_sb[:], fc2_sb[:])
            h = {}
            for s in (0, 1):
                bc = bcast(st1["r_r" if s == 0 else "r_d"], f"n1_{g}_{s}")
                for j in range(CT):
                    ht = hp.tile([128, TG], dt.bfloat16, tag="h",
                                 name=f"h1_{g}_{s}_{j}")
                    nc.vector.tensor_tensor(ht[:], x[s, j][:], bc[0:128, :],
                                            Alu.mult)
                    h[s, j] = ht
            # k-outer into 6 psum banks: PE starts on the first h tile.
            # g_r lands first, is consumed by the x1_d residuals (swap),
            # then the banks recycle for g_d -> x1_r.
            for s, o in ((0, 1), (1, 0)):
                accs = []
                for mo in range(CT):
                    a_ = psum.tile([128, TG], dt.float32, tag="acc", bufs=6,
                                   name=f"g_{g}_{s}_{mo}")
                    accs.append(a_)
                for k in range(CT):
                    for mo in range(CT):
                        nc.tensor.matmul(
                            accs[mo][:],
                            wc_sb[:, k * C + mo * 128 : k * C + (mo + 1) * 128],
                            h[s, k][:], start=(k == 0), stop=False)
                mr = st1["mr_r" if s == 0 else "mr_d"]
                for mo in range(CT):
                    nc.tensor.matmul(
                        accs[mo][:],
                        wcsum_sb[0:1, mo * 128 : (mo + 1) * 128],
                        mr, start=False, stop=True)
                    # x1_o = x0_o + g_s + pb (in place), o = other stream
                    nc.vector.scalar_tensor_tensor(x[o, mo][:], accs[mo][:],
                                                   vec(V_PB, mo), x[o, mo][:],
                                                   Alu.add, Alu.add)

            # ---- norm2 + MLP ----
            st2 = ln_stats([x[0, j] for j in range(CT)],
                           [x[1, j] for j in range(CT)], f"n2_{g}")
            h2 = {}
            for s in (0, 1):
                bc = bcast(st2["r_r" if s == 0 else "r_d"], f"n2_{g}_{s}")
                # broadcast of m*r (bf16 rank-1): mr' = S1*rr = m*r*sqrt(C),
                # scaled back by the 1/sqrt(C)-valued lhsT row
                bcm = psum.tile([128, TG], dt.float32, tag="ps",
                                name=f"bcm_n2_{g}_{s}")
                nc.tensor.matmul(bcm[:], isqrtc_row_b[:],
                                 st2["mr_r" if s == 0 else "mr_d"],
                                 start=True, stop=True)
                bcm_sb = tmpp.tile([128, TG], dt.float32, tag="bcmsb", bufs=2,
                                   name=f"bcmsb_{g}_{s}")
                nc.scalar.copy(bcm_sb[:], bcm[:])
                for j in range(CT):
                    t_ = tmpp.tile([128, TG], dt.float32, tag="bl",
                                   name=f"h2t_{g}_{s}_{j}")
                    nc.vector.tensor_tensor(t_[:], x[s, j][:], bc[0:128, :],
                                            Alu.mult)
                    ht = hp.tile([128, TG], dt.bfloat16, tag="h",
                                 name=f"h2_{g}_{s}_{j}")
                    nc.gpsimd.tensor_tensor(ht[:], t_[:], bcm_sb[0:128, :],
                                            Alu.subtract)
                    h2[s, j] = ht
            for s in (0, 1):
                acc = []
                for co in range(CT):
                    a_ = psum.tile([128, TG], dt.float32, tag="acc", bufs=6,
                                   name=f"acc_{g}_{s}_{co}")
                    acc.append(a_)
                for m in range(MT):
                    pf = psum.tile([128, TG], dt.float32, tag="ps",
                                   name=f"pf_{g}_{s}_{m}")
                    for k in range(CT):
                        nc.tensor.matmul(
                            pf[:],
                            fc1_sb[:, k * MLP + m * 128 : k * MLP + (m + 1) * 128],
                            h2[s, k][:], start=(k == 0), stop=(k == CT - 1))
                    am = apool.tile([128, TG], dt.bfloat16, tag="a",
                                    name=f"a_{g}_{s}_{m}")
                    nc.scalar.activation(am[:], pf[:], act_fn,
                                         bias=fb1_sb[:, m : m + 1], scale=1.0)
                    for co in range(CT):
                        nc.tensor.matmul(
                            acc[co][:],
                            fc2_sb[:, m * C + co * 128 : m * C + (co + 1) * 128],
                            am[:], start=(m == 0), stop=(m == MT - 1))
                for co in range(CT):
                    nc.vector.scalar_tensor_tensor(x[s, co][:], acc[co][:],
                                                   vec(V_FC2B, co), x[s, co][:],
                                                   Alu.add, Alu.add)

            # ---- final norm + modality mean + transpose out ----
            stf = ln_stats([x[0, j] for j in range(CT)],
                           [x[1, j] for j in range(CT)], f"nf_{g}")
            bc_rr = bcast(stf["r_r"], f"nf_{g}_r")
            bc_rd = bcast(stf["r_d"], f"nf_{g}_d")
            # broadcast of (mr_r + mr_d): two accumulated K=1 ones matmuls
            bc_mrs = psum.tile([128, TG], dt.float32, tag="acc", bufs=6,
                               name=f"bcmrs_{g}")
            nc.tensor.matmul(bc_mrs[:], isqrtc_row_b[:],
                             stf["mr_r"], start=True, stop=False)
            nc.tensor.matmul(bc_mrs[:], isqrtc_row_b[:],
                             stf["mr_d"], start=False, stop=True)
            uas = []
            for j in range(CT):
                s1 = tmpp.tile([128, TG], dt.float32, tag="bl",
                               name=f"nf1_{g}_{j}")
                nc.vector.tensor_tensor(s1[:], x[0, j][:], bc_rr[0:128, :],
                                        Alu.mult)
                s2 = tmpp.tile([128, TG], dt.float32, tag="bl",
                               name=f"nf2_{g}_{j}")
                nc.vector.tensor_tensor(s2[:], x[1, j][:], bc_rd[0:128, :],
                                        Alu.mult)
                nc.gpsimd.tensor_tensor(s1[:], s1[:], s2[:], Alu.add)
                nc.vector.tensor_tensor(s1[:], s1[:], bc_mrs[0:128, :],
                                        Alu.subtract)
                ua = uaffp.tile([128, TG], dt.float32, tag="uaff",
                                name=f"ua_{g}_{j}")
                nc.scalar.activation(ua[:], s1[:], Act.Identity,
                                     bias=vec(V_BF, j), scale=vec(V_WFH, j))
                uas.append(ua)
            for tt in range(NTT):
                po = psum.tile([128, 512], dt.float32, tag="acc", bufs=6,
                               name=f"po_{g}_{tt}")
                po2 = psum.tile([128, 512], dt.float32, tag="acc", bufs=6,
                                name=f"po2_{g}_{tt}")
                for j in range(CT):
                    dst = (po[:, j * 128 : (j + 1) * 128] if j < 4
                           else po2[:, (j - 4) * 128 : (j - 3) * 128])
                    nc.tensor.transpose(
                        dst, uas[j][:, tt * 128 : (tt + 1) * 128], ident_sb[:])
                ot = outp.tile([128, C], dt.float32, tag="ot",
                               name=f"ot_{g}_{tt}")
                nc.scalar.copy(ot[:, 0:512], po[:, :])
                nc.scalar.copy(ot[:, 512:768], po2[:, 0:256])
                nc.sync.dma_start(
                    out_d[r0 + tt * 128 : r0 + (tt + 1) * 128, :], ot[:])

    if legalize:
        _legalize_waits(nc)
    nc.finalize()
    return nc


def _legalize_waits(nc):
    """Walrus ISA structs have at most 1-2 sync-wait slots per instruction,
    but Tile's wait assignment can emit more. Move excess waits onto
    same-engine NoOps inserted immediately before the offending instruction
    (engines execute their stream in order, so an earlier wait on the same
    engine is equivalent)."""
    import bass_rust
    nop_i = [0]
    for f in nc.m.functions:
        for b in f.blocks:
            insts = b.instructions
            out = []
            changed = False
            for ins in insts:
                si = getattr(ins, "sync_info", None)
                waits = list(si.on_wait) if (si and si.on_wait) else []
                if len(waits) > 1:
                    eng = ins.engine
                    for w in waits[:-1]:
                        n = bass_rust.InstNoOp(name=f"I-nopw-{nop_i[0]}")
                        nop_i[0] += 1
                        n.engine = eng
                        n.sync_info = bass_rust.SyncInfo(
                            on_wait=[w], on_update=[])
                        out.append(n)
                    ins.sync_info = bass_rust.SyncInfo(
                        on_wait=[waits[-1]], on_update=list(si.on_update or []))
                    changed = True
                out.append(ins)
            if changed:
                b.instructions = out


def _prepare(inputs):
    """Host-side folding: per-channel vectors + fused/packed weights."""
    f = lambda k: np.asarray(inputs[k], np.float64)
    alpha = f("alpha").reshape(C)

    s_r = f("bn_rgb_w") / np.sqrt(f("bn_rgb_var") + EPS)
    t_r = f("bn_rgb_b") - f("bn_rgb_mean") * s_r
    s_d = f("bn_depth_w") / np.sqrt(f("bn_depth_var") + EPS)
    t_d = f("bn_depth_b") - f("bn_depth_mean") * s_d

    w_r = np.asarray(inputs["bn_rgb_w"], np.float32)
    w_d = np.asarray(inputs["bn_depth_w"], np.float32)
    idx_r = np.argsort(np.abs(w_r), kind="stable")[:K_EX]
    idx_d = np.argsort(np.abs(w_d), kind="stable")[:K_EX]
    mask_r = np.zeros(C, bool)
    mask_r[idx_r] = True
    mask_d = np.zeros(C, bool)
    mask_d[idx_d] = True

    A1 = np.where(mask_r, alpha * s_r, s_r)
    A2 = np.where(mask_r, (1 - alpha) * s_d, 0.0)
    A3 = np.where(mask_r, alpha * t_r + (1 - alpha) * t_d, t_r)
    D1 = np.where(mask_d, alpha * s_d, s_d)
    D2 = np.where(mask_d, (1 - alpha) * s_r, 0.0)
    D3 = np.where(mask_d, alpha * t_d + (1 - alpha) * t_r, t_d)

    qkv_w = f("qkv_w")
    Wv = qkv_w[2 * C :, :]
    Wc = f("proj_w") @ Wv
    w1, b1 = f("norm1_w"), f("norm1_b")
    Wc_f = Wc * w1[None, :]
    pb = f("proj_b") + Wc @ b1
    wc_rowsum = Wc_f.sum(axis=1)

    w2, b2 = f("norm2_w"), f("norm2_b")
    fc1_f = f("fc1_w") * w2[None, :]
    fb1 = f("fc1_b") + f("fc1_w") @ b2
    fc1_rowsum = fc1_f.sum(axis=1)
    fc2_w = f("fc2_w")
    fc2_b = f("fc2_b")
    wfh = 0.5 * f("normf_w")
    bf_ = f("normf_b")

    bf16 = ml_dtypes.bfloat16

    def pack_lhsT(wT, kt, m):
        # wT: [kt*128, m]  ->  [128, kt*m] with [p, k*m + col] = wT[128k+p, col]
        return np.ascontiguousarray(
            wT.reshape(kt, 128, m).transpose(1, 0, 2).reshape(128, kt * m))

    wc_pack = pack_lhsT(np.ascontiguousarray(Wc_f.T), CT, C).astype(bf16)
    fc1_pack = pack_lhsT(np.ascontiguousarray(fc1_f.T), CT, MLP).astype(bf16)
    fc2_pack = pack_lhsT(np.ascontiguousarray(fc2_w.T), MT, C).astype(bf16)

    vv = [A1, A2, A3, D1, D2, D3, pb, fc2_b, wfh, bf_]
    vecs = np.stack(vv, axis=-1).astype(np.float32)          # [C, NV]
    vecs = vecs.reshape(CT, 128, NV).transpose(1, 0, 2).reshape(128, CT * NV)
    vecs = np.ascontiguousarray(vecs)
    fb1_pack = np.ascontiguousarray(
        fb1.astype(np.float32).reshape(MT, 128).T)           # [128, MT]

    return {
        "wc": wc_pack,
        "fc1": fc1_pack,
        "fc2": fc2_pack,
        "vecs": vecs,
        "fb1": fb1_pack,
        "wcsum": (-wc_rowsum / np.sqrt(C)).astype(bf16).reshape(1, C),
        "fc1sum": (-fc1_rowsum / np.sqrt(C)).astype(bf16).reshape(1, MLP),
        "ident": np.eye(128, dtype=np.float32),
    }


def _get_runner():
    """Build the Bass module once and cache a jitted shard_map executor so
    repeat kernel() calls skip jax retracing / PJRT re-compilation."""
    if "runner" in _CACHE:
        return _CACHE["runner"]
    import jax
    from jax.sharding import Mesh, PartitionSpec
    from jax.experimental.shard_map import shard_map
    from concourse import bass2jax

    nc = _build_nc()
    bass2jax.install_neuronx_cc_hook()
    partition_name = (nc.partition_id_tensor.name
                      if nc.partition_id_tensor else None)
    in_names, out_names, out_avals = [], [], []
    for alloc in nc.m.functions[0].allocations:
        if not isinstance(alloc, mybir.MemoryLocationSet):
            continue
        name = alloc.memorylocations[0].name
        if alloc.kind == "ExternalInput":
            if name != partition_name:
                in_names.append(name)
        elif alloc.kind == "ExternalOutput":
            out_names.append(name)
            out_avals.append(jax.core.ShapedArray(
                tuple(alloc.tensor_shape), mybir.dt.np(alloc.dtype)))
    n_params = len(in_names)
    all_in_names = list(in_names) + list(out_names)
    if partition_name is not None:
        all_in_names.append(partition_name)

    def _body(*args):
        operands = list(args)
        if partition_name is not None:
            operands.append(bass2jax.partition_id_tensor())
        return tuple(bass2jax._bass_exec_p.bind(
            *operands, out_avals=tuple(out_avals),
            in_names=tuple(all_in_names), out_names=tuple(out_names),
            lowering_input_output_aliases=(),
            sim_require_finite=True, sim_require_nnan=True, nc=nc))

    devices = jax.devices()[:N_CORES]
    mesh = Mesh(np.asarray(devices), ("core",))
    sharded_args = {"rgb", "dep"}
    in_specs = tuple(
        PartitionSpec("core") if n in sharded_args else PartitionSpec()
        for n in in_names) + (PartitionSpec("core"),) * len(out_names)
    fn = jax.jit(
        shard_map(_body, mesh=mesh,
                  in_specs=in_specs,
                  out_specs=(PartitionSpec("core"),) * len(out_names),
                  check_rep=False),
        keep_unused=True)
    zeros = [jax.device_put(
        np.zeros((a.shape[0] * N_CORES,) + tuple(a.shape[1:]), a.dtype))
        for a in out_avals]
    _CACHE["runner"] = (fn, in_names, zeros, jax)
    return _CACHE["runner"]


def kernel(**inputs) -> np.ndarray:
    rgb = np.ascontiguousarray(np.asarray(inputs["rgb"], np.float32))
    dep = np.ascontiguousarray(np.asarray(inputs["depth"], np.float32))
    consts = _prepare(inputs)

    fn, in_names, zeros, jax = _get_runner()
    vals = {
        "rgb": rgb.reshape(ROWS * N_CORES, C),
        "dep": dep.reshape(ROWS * N_CORES, C),
    }
    # constant tensors are replicated (PartitionSpec()) - ship one copy
    vals.update(consts)
    args = [vals[n] for n in in_names] + list(zeros)
    outs = fn(*args)
    out = np.asarray(outs[0]).reshape(B, T, C)
    return out


if __name__ == "__main__":
    rng = np.random.default_rng(0)
    demo = {
        "rgb": rng.standard_normal((B, T, C), np.float32),
        "depth": rng.standard_normal((B, T, C), np.float32),
    }
    print("built module ok" if _build_nc() else "")



# revision 6
# speedup vs baseline: 1.9124x; 1.9124x over previous
"""Trainium2 Bass kernel for nn_CMFuser (topk_masking) — v2.

Self-contained: accepts FULL inputs (as produced by setup_inputs()), returns
the FULL [32, 512, 768] output. Internally shards batch across 8 NeuronCores
(pure data parallel, 4 batches/core) and runs a hand-written Bass/Tile kernel.

Algorithmic structure (same folding as v1, validated vs the jax reference):
  * BN(eval) + topk-channel-exchange blend folds into per-channel affine:
        x0_rgb = A1*rgb + A2*depth + A3,   x0_depth = D1*depth + D2*rgb + D3
  * The 2-token attention with -1e9 diag mask is an EXACT token swap, so
    qkv+softmax+proj collapse into one fused C x C matmul Wc = proj_w @ Wv
    applied to the OTHER token.
  * LN weights fold into the following matmul; LN mean-subtraction folds
    into a rank-1 (K=1) matmul correction (norm1) / bcast subtract (norm2).
  * Final LN + mean over the 2 modality tokens folds into 0.5*wf scale.

v2 performance changes vs the 895us baseline:
  * fc1/fc2 matmuls in fp8e4m3 with DoubleRow perf mode (0.5 cyc/row and
    K=256 per instruction -> 4x the bf16 PE throughput). Weights are
    pre-scaled by 16 before quantization to dodge the e4m3 subnormal range;
    the 1/16 descale folds into the GELU input scale and the fc2-residual
    per-channel multiplier. Wc stays bf16 (fp8 there pushes rel-err > 2e-2).
  * Whole residual stream, inputs, transposes and LN stats in fp16:
    - input DMA traffic halved (host casts rgb/depth to fp16),
    - PE transposes at 1 cyc/row (fp16 identity),
    - DVE elementwise ops at 2x (2-byte dtypes).
  * Elementwise work spread across DVE / ACT / Pool by measured budget.
Measured error of this exact quantization recipe vs the reference
(numpy emulation): 1.45e-2 < 2e-2.
"""

import os
import sys

sys.path.insert(0, "/opt/trn_rl_repo")

import numpy as np
import ml_dtypes

import concourse.bass as bass
import concourse.mybir as mybir
import concourse.tile as tile
from contextlib import ExitStack

dt = mybir.dt
Alu = mybir.AluOpType
Act = mybir.ActivationFunctionType
PerfMode = mybir.MatmulPerfMode

B, T, C = 32, 512, 768
H = 4
K_EX = int(C * 0.2)
MLP = 4 * C
EPS = 1e-5
N_CORES = 8
B_CORE = B // N_CORES          # 4 batches per core
ROWS = B_CORE * T              # 2048 token-sites per core
TG = 512                       # tokens per group
NG = ROWS // TG                # 4 groups per core
CT = C // 128                  # 6 channel tiles
CP = CT // 2                   # 3 channel k-pairs (DoubleRow)
MT = MLP // 128                # 24 mlp tiles
MP = MT // 2                   # 12 mlp k-pairs
NTT = TG // 128                # 4 token tiles per group
WSCALE = 16.0                  # fp8 weight pre-scale (descaled on device)

# vector slot indices in the packed per-channel constant table
V_A1, V_A2, V_A3, V_D1, V_D2, V_D3, V_PB, V_SCL, V_WFH, V_BF = range(10)
NV = 10

_CACHE = {}


def _build_nc(legalize=True):
    """Build the per-core Bass module (same program on all 8 cores)."""
    nc = bass.Bass()

    rgb_d = nc.dram_tensor("rgb", [ROWS, C], dt.float16, kind="ExternalInput")
    dep_d = nc.dram_tensor("dep", [ROWS, C], dt.float16, kind="ExternalInput")
    wc_d = nc.dram_tensor("wc", [128, CT * C], dt.bfloat16, kind="ExternalInput")
    fc1_d = nc.dram_tensor("fc1", [128, CP * 2 * MLP], dt.float8e4,
                           kind="ExternalInput")
    fc2_d = nc.dram_tensor("fc2", [128, MP * 2 * C], dt.float8e4,
                           kind="ExternalInput")
    vecs_d = nc.dram_tensor("vecs", [128, CT * NV], dt.float32,
                            kind="ExternalInput")
    fb1_d = nc.dram_tensor("fb1", [128, MT], dt.float32, kind="ExternalInput")
    wcsum_d = nc.dram_tensor("wcsum", [1, C], dt.bfloat16, kind="ExternalInput")
    ident_d = nc.dram_tensor("ident", [128, 128], dt.float16,
                             kind="ExternalInput")
    out_d = nc.dram_tensor("out", [ROWS, C], dt.float32, kind="ExternalOutput")

    with tile.TileContext(nc) as tc, ExitStack() as ctx:
        const = ctx.enter_context(tc.tile_pool(name="const", bufs=1))
        inp = ctx.enter_context(tc.tile_pool(name="inp", bufs=16))
        xp = ctx.enter_context(tc.tile_pool(name="xp", bufs=16))
        sqp = ctx.enter_context(tc.tile_pool(name="sqp", bufs=6))
        h1p = ctx.enter_context(tc.tile_pool(name="h1p", bufs=14))
        h2p = ctx.enter_context(tc.tile_pool(name="h2p", bufs=8))
        a8p = ctx.enter_context(tc.tile_pool(name="a8p", bufs=5))
        bcp = ctx.enter_context(tc.tile_pool(name="bcp", bufs=8))
        tmpp = ctx.enter_context(tc.tile_pool(name="tmpp", bufs=6))
        rows = ctx.enter_context(tc.tile_pool(name="rows", bufs=10))
        rows1 = ctx.enter_context(tc.tile_pool(name="rows1", bufs=8))
        uap = ctx.enter_context(tc.tile_pool(name="uap", bufs=8))
        outp = ctx.enter_context(tc.tile_pool(name="outp", bufs=3))
        psum = ctx.enter_context(tc.tile_pool(name="psum", bufs=2, space="PSUM"))

        # ---- constants / weights (small, early-needed tensors first) ----
        ident_sb = const.tile([128, 128], dt.float16)
        nc.sync.dma_start(ident_sb[:], ident_d[:])
        vecs_sb = const.tile([128, CT * NV], dt.float32)
        nc.sync.dma_start(vecs_sb[:], vecs_d[:])
        fb1_sb = const.tile([128, MT], dt.float32)
        nc.sync.dma_start(fb1_sb[:], fb1_d[:])
        wcsum_sb = const.tile([1, C], dt.bfloat16)
        nc.sync.dma_start(wcsum_sb[:], wcsum_d[:])
        # group-0 inputs issued BEFORE the big weight loads
        pre_in = {}
        for s_, src_ in ((0, rgb_d), (1, dep_d)):
            for tt_ in range(NTT):
                it_ = inp.tile([128, C], dt.float16, tag="in",
                               name=f"in_0_{s_}_{tt_}")
                nc.sync.dma_start(it_[:], src_[tt_ * 128:(tt_ + 1) * 128, :])
                pre_in[s_, tt_] = it_
        wc_sb = const.tile([128, CT * C], dt.bfloat16)
        nc.sync.dma_start(wc_sb[:], wc_d[:])
        fc1_sb = const.tile([128, CP, 2, MLP], dt.float8e4)
        nc.sync.dma_start(fc1_sb[:, :, :, :], fc1_d[:, :])
        fc2_sb = const.tile([128, MP, 2, C], dt.float8e4)
        nc.sync.dma_start(fc2_sb[:, :, :, :], fc2_d[:, :])

        ones16 = const.tile([128, 1], dt.float16)
        nc.vector.memset(ones16[:], 1.0)
        sqrtc_f32 = const.tile([1, 128], dt.float32)
        nc.vector.memset(sqrtc_f32[:], float(np.sqrt(C)))
        sqrtc_row = const.tile([1, 128], dt.float32r)
        with nc.allow_low_precision("fp32r bcast lhsT"):
            nc.vector.tensor_copy(sqrtc_row[:], sqrtc_f32[:])
        isqrtc_row_b = const.tile([1, 128], dt.bfloat16)
        nc.vector.memset(isqrtc_row_b[:], float(1.0 / np.sqrt(C)))
        ceps_ap = const.tile([1, 1], dt.float32)
        nc.vector.memset(ceps_ap[:], float(C * EPS))

        def vec(idx, j):
            return vecs_sb[:, j * NV + idx: j * NV + idx + 1]

        # per-group state dicts
        in_tiles = [None] * NG
        x = [None] * NG            # (s, j) -> [128,TG] fp16 residual tiles
        st_rows = [None] * NG      # per-LN row results
        h1 = [None] * NG
        h2pair = [None] * NG

        def stage_L(g):
            """input DMA (g>0), PE transpose, DVE blend -> x0."""
            if g == 0:
                in_tiles[g] = pre_in
            else:
                it_map = {}
                r0 = g * TG
                for s, src in ((0, rgb_d), (1, dep_d)):
                    for tt_ in range(NTT):
                        it = inp.tile([128, C], dt.float16, tag="in",
                                      name=f"in_{g}_{s}_{tt_}")
                        nc.sync.dma_start(
                            it[:], src[r0 + tt_ * 128: r0 + (tt_ + 1) * 128, :])
                        it_map[s, tt_] = it
                in_tiles[g] = it_map
            xg = {}
            for j in range(CT):
                pt = {}
                for s in (0, 1):
                    p = psum.tile([128, TG], dt.float16, tag="acc", bufs=6,
                                  padded_shape=[128, 1024],
                                  name=f"pt_{g}_{s}_{j}")
                    for tt_ in range(NTT):
                        nc.tensor.transpose(
                            p[:, tt_ * 128:(tt_ + 1) * 128],
                            in_tiles[g][s, tt_][:, j * 128:(j + 1) * 128],
                            ident_sb[:])
                    pt[s] = p
                t1 = tmpp.tile([128, TG], dt.float16, tag="bl",
                               name=f"t1_{g}_{j}")
                nc.vector.tensor_scalar(t1[:], pt[1][:], vec(V_A2, j),
                                        vec(V_A3, j), Alu.mult, Alu.add)
                x0r = xp.tile([128, TG], dt.float16, tag="res",
                              name=f"x0r_{g}_{j}")
                nc.vector.scalar_tensor_tensor(x0r[:], pt[0][:], vec(V_A1, j),
                                               t1[:], Alu.mult, Alu.add)
                t2 = tmpp.tile([128, TG], dt.float16, tag="bl",
                               name=f"t2_{g}_{j}")
                nc.vector.tensor_scalar(t2[:], pt[0][:], vec(V_D2, j),
                                        vec(V_D3, j), Alu.mult, Alu.add)
                x0d = xp.tile([128, TG], dt.float16, tag="res",
                              name=f"x0d_{g}_{j}")
                nc.vector.scalar_tensor_tensor(x0d[:], pt[1][:], vec(V_D1, j),
                                               t2[:], Alu.mult, Alu.add)
                xg[0, j] = x0r
                xg[1, j] = x0d
            x[g] = xg

        def ln_stats(g, name, sq_engine="vector"):
            """LN stats over channels for both streams of group g.

            Returns {('r'|'d'): (rrow_f32r, mr_bf16)}; rows are [1,TG].
            PE: 12 fp16 matmuls per stream into one psum stat tile
            (sum at partition 0, sum-of-squares at partition 32).
            """
            out = {}
            for s in (0, 1):
                sfx = "r" if s == 0 else "d"
                sq = []
                for j in range(CT):
                    sqt = sqp.tile([128, TG], dt.float16, tag="sq",
                                   name=f"sq_{name}_{s}_{j}")
                    if sq_engine == "vector":
                        nc.vector.tensor_tensor(sqt[:], x[g][s, j][:],
                                                x[g][s, j][:], Alu.mult)
                    elif sq_engine == "pool":
                        nc.gpsimd.tensor_tensor(sqt[:], x[g][s, j][:],
                                                x[g][s, j][:], Alu.mult)
                    else:
                        nc.scalar.square(sqt[:], x[g][s, j][:])
                    sq.append(sqt)
                stat = psum.tile([128, TG], dt.float32, tag="acc", bufs=6,
                                 name=f"stat_{name}_{s}")
                for j in range(CT):
                    nc.tensor.matmul(stat[0:1, :], ones16[:], x[g][s, j][:],
                                     tile_position=(0, 0),
                                     start=(j == 0), stop=(j == CT - 1))
                    nc.tensor.matmul(stat[32:33, :], ones16[:], sq[j][:],
                                     tile_position=(0, 32),
                                     start=(j == 0), stop=(j == CT - 1))
                sq1 = rows.tile([1, TG], dt.float32, tag="rows",
                                name=f"sq1_{name}_{s}")
                nc.scalar.square(sq1[:], stat[0:1, :])
                u = rows.tile([1, TG], dt.float32, tag="rows",
                              name=f"u_{name}_{s}")
                nc.vector.scalar_tensor_tensor(u[:], sq1[:], -1.0 / C,
                                               stat[32:33, :], Alu.mult,
                                               Alu.add)
                std = rows.tile([1, TG], dt.float32, tag="rows",
                                name=f"std_{name}_{s}")
                nc.scalar.activation(std[:], u[:], Act.Sqrt,
                                     bias=ceps_ap[0:1, 0:1], scale=1.0)
                rrow = rows1.tile([1, TG], dt.float32r, tag="rows1", bufs=4,
                                  name=f"r_{name}_{s}")
                with nc.allow_low_precision("fp32r bcast rows"):
                    nc.vector.reciprocal(rrow[:], std[:])
                mr = rows1.tile([1, TG], dt.bfloat16, tag="rows1b", bufs=4,
                                name=f"mr_{name}_{s}")
                nc.vector.tensor_tensor(mr[:], stat[0:1, :], rrow[:], Alu.mult)
                out[sfx] = (rrow, mr)
            return out

        def bcast_r16(rrow, name):
            """rsqrt row broadcast: K=1 PE matmul + ACT copy to fp16 SBUF."""
            bc = psum.tile([128, TG], dt.float32, tag="ps", name=f"bc_{name}")
            nc.tensor.matmul(bc[:], sqrtc_row[0:1, :], rrow[:],
                             start=True, stop=True)
            bc16 = bcp.tile([128, TG], dt.float16, tag="bc16",
                            name=f"bc16_{name}")
            nc.scalar.copy(bc16[:], bc[:])
            return bc16

        def stage_W(g):
            """norm1 apply + Wc (attention swap) + residual -> x1."""
            st = st_rows[g]
            h1g = {}
            for s in (0, 1):
                sfx = "r" if s == 0 else "d"
                bc16 = bcast_r16(st[sfx][0], f"n1_{g}_{s}")
                for j in range(CT):
                    ht = h1p.tile([128, TG], dt.bfloat16, tag="h1",
                                  name=f"h1_{g}_{s}_{j}")
                    nc.vector.tensor_tensor(ht[:], x[g][s, j][:], bc16[:],
                                            Alu.mult)
                    h1g[s, j] = ht
            h1[g] = h1g
            for s, o in ((0, 1), (1, 0)):
                accs = []
                for mo in range(CT):
                    a_ = psum.tile([128, TG], dt.float32, tag="acc", bufs=6,
                                   name=f"g_{g}_{s}_{mo}")
                    accs.append(a_)
                for k in range(CT):
                    for mo in range(CT):
                        nc.tensor.matmul(
                            accs[mo][:],
                            wc_sb[:, k * C + mo * 128: k * C + (mo + 1) * 128],
                            h1g[s, k][:], start=(k == 0), stop=False)
                mr = st["r" if s == 0 else "d"][1]
                for mo in range(CT):
                    nc.tensor.matmul(
                        accs[mo][:],
                        wcsum_sb[0:1, mo * 128:(mo + 1) * 128],
                        mr[:], start=False, stop=True)
                    # x1_o = x0_o + g_s + pb (in place), o = other stream
                    # (gpsimd cannot read PSUM -> DVE)
                    nc.vector.scalar_tensor_tensor(x[g][o, mo][:], accs[mo][:],
                                                   vec(V_PB, mo),
                                                   x[g][o, mo][:],
                                                   Alu.add, Alu.add)

        def stage_M(g):
            """norm2 apply + fp8 DoubleRow MLP + residual -> x2."""
            st = st_rows[g]
            h2g = {}
            for s in (0, 1):
                sfx = "r" if s == 0 else "d"
                rrow, mr = st[sfx]
                bc16 = bcast_r16(rrow, f"n2_{g}_{s}")
                bcm = psum.tile([128, TG], dt.float32, tag="ps",
                                name=f"bcm_{g}_{s}")
                nc.tensor.matmul(bcm[:], isqrtc_row_b[0:1, :], mr[:],
                                 start=True, stop=True)
                bcm16 = bcp.tile([128, TG], dt.float16, tag="bc16",
                                 name=f"bcm16_{g}_{s}")
                nc.scalar.copy(bcm16[:], bcm[:])
                for kp in range(CP):
                    pair = h2p.tile([128, 2, TG], dt.float8e4, tag="h2",
                                    name=f"h2_{g}_{s}_{kp}")
                    for i in (0, 1):
                        j = 2 * kp + i
                        t_ = tmpp.tile([128, TG], dt.float16, tag="bl",
                                       name=f"h2t_{g}_{s}_{j}")
                        nc.vector.tensor_tensor(t_[:], x[g][s, j][:], bc16[:],
                                                Alu.mult)
                        nc.gpsimd.tensor_tensor(pair[:, i, :], t_[:],
                                                bcm16[:], Alu.subtract)
                    h2g[s, kp] = pair
            for s in (0, 1):
                acc2 = []
                for co in range(CT):
                    a_ = psum.tile([128, TG], dt.float32, tag="acc", bufs=6,
                                   name=f"acc_{g}_{s}_{co}")
                    acc2.append(a_)
                apair = None
                for m in range(MT):
                    pf = psum.tile([128, TG], dt.float32, tag="ps",
                                   name=f"pf_{g}_{s}_{m}")
                    for kp in range(CP):
                        nc.tensor.matmul(
                            pf[:],
                            fc1_sb[:, kp, :, m * 128:(m + 1) * 128],
                            h2g[s, kp][:, :, :],
                            start=(kp == 0), stop=(kp == CP - 1),
                            perf_mode=PerfMode.DoubleRow)
                    if m % 2 == 0:
                        apair = a8p.tile([128, 2, TG], dt.float8e4, tag="a8",
                                         name=f"a_{g}_{s}_{m // 2}")
                    nc.scalar.activation(apair[:, m % 2, :], pf[:], Act.Gelu,
                                         bias=fb1_sb[:, m:m + 1],
                                         scale=float(1.0 / WSCALE))
                    if m % 2 == 1:
                        mp = m // 2
                        for co in range(CT):
                            nc.tensor.matmul(
                                acc2[co][:],
                                fc2_sb[:, mp, :, co * 128:(co + 1) * 128],
                                apair[:, :, :],
                                start=(mp == 0), stop=(mp == MP - 1),
                                perf_mode=PerfMode.DoubleRow)
                for co in range(CT):
                    # x2 = acc/WSCALE + x1 (fc2_b guaranteed zero; V_SCL=1/16)
                    nc.vector.scalar_tensor_tensor(x[g][s, co][:],
                                                   acc2[co][:],
                                                   vec(V_SCL, co),
                                                   x[g][s, co][:],
                                                   Alu.mult, Alu.add)

        def stage_F(g):
            """final norm + modality mean + transpose out + DMA."""
            st = st_rows[g]
            bc_rr16 = bcast_r16(st["r"][0], f"nf_{g}_r")
            bc_rd16 = bcast_r16(st["d"][0], f"nf_{g}_d")
            bc_mrs = psum.tile([128, TG], dt.float32, tag="ps",
                               name=f"bcmrs_{g}")
            nc.tensor.matmul(bc_mrs[:], isqrtc_row_b[0:1, :], st["r"][1][:],
                             start=True, stop=False)
            nc.tensor.matmul(bc_mrs[:], isqrtc_row_b[0:1, :], st["d"][1][:],
                             start=False, stop=True)
            bc_mrs16 = bcp.tile([128, TG], dt.float16, tag="bc16",
                                name=f"bcmrs16_{g}")
            nc.scalar.copy(bc_mrs16[:], bc_mrs[:])
            uas = []
            for j in range(CT):
                s1 = tmpp.tile([128, TG], dt.float16, tag="bl",
                               name=f"nf1_{g}_{j}")
                nc.vector.tensor_tensor(s1[:], x[g][0, j][:], bc_rr16[:],
                                        Alu.mult)
                s2 = tmpp.tile([128, TG], dt.float16, tag="bl",
                               name=f"nf2_{g}_{j}")
                nc.vector.tensor_tensor(s2[:], x[g][1, j][:], bc_rd16[:],
                                        Alu.mult)
                nc.gpsimd.tensor_tensor(s1[:], s1[:], s2[:], Alu.add)
                nc.vector.tensor_tensor(s1[:], s1[:], bc_mrs16[:],
                                        Alu.subtract)
                ua = uap.tile([128, TG], dt.float16, tag="uaff",
                              name=f"ua_{g}_{j}")
                nc.vector.tensor_scalar(ua[:], s1[:], vec(V_WFH, j),
                                        vec(V_BF, j), Alu.mult, Alu.add)
                uas.append(ua)
            r0 = g * TG
            for tt_ in range(NTT):
                po = psum.tile([128, TG], dt.float16, tag="acc", bufs=6,
                               padded_shape=[128, 1024], name=f"po_{g}_{tt_}")
                po2 = psum.tile([128, TG], dt.float16, tag="acc", bufs=6,
                                padded_shape=[128, 1024],
                                name=f"po2_{g}_{tt_}")
                for j in range(CT):
                    dst = (po[:, j * 128:(j + 1) * 128] if j < 4
                           else po2[:, (j - 4) * 128:(j - 3) * 128])
                    nc.tensor.transpose(
                        dst, uas[j][:, tt_ * 128:(tt_ + 1) * 128], ident_sb[:])
                ot = outp.tile([128, C], dt.float32, tag="ot",
                               name=f"ot_{g}_{tt_}")
                nc.scalar.copy(ot[:, 0:512], po[:, :])
                nc.scalar.copy(ot[:, 512:768], po2[:, 0:256])
                nc.sync.dma_start(
                    out_d[r0 + tt_ * 128: r0 + (tt_ + 1) * 128, :], ot[:])

        # ---- software-pipelined schedule over groups ----
        def S1(g):
            st_rows[g] = ln_stats(g, f"n1_{g}", sq_engine="pool")

        def S2(g):
            st_rows[g] = ln_stats(g, f"n2_{g}", sq_engine="pool")

        def SF(g):
            st_rows[g] = ln_stats(g, f"nf_{g}", sq_engine="scalar")

        sched = []
        for g in range(NG):
            sched += [(stage_L, g), (S1, g), (stage_W, g), (S2, g),
                      (stage_M, g), (SF, g), (stage_F, g)]
        for fn, g in sched:
            fn(g)

    if legalize:
        _legalize_waits(nc)
    nc.finalize()
    return nc


def _legalize_waits(nc):
    """Walrus ISA structs have at most 1-2 sync-wait slots per instruction,
    but Tile's wait assignment can emit more. Move excess waits onto
    same-engine NoOps inserted immediately before the offending
    instruction."""
    import bass_rust
    nop_i = [0]
    for f in nc.m.functions:
        for b in f.blocks:
            insts = b.instructions
            out = []
            changed = False
            for ins in insts:
                si = getattr(ins, "sync_info", None)
                waits = list(si.on_wait) if (si and si.on_wait) else []
                if len(waits) > 1:
                    eng = ins.engine
                    for w in waits[:-1]:
                        n = bass_rust.InstNoOp(name=f"I-nopw-{nop_i[0]}")
                        nop_i[0] += 1
                        n.engine = eng
                        n.sync_info = bass_rust.SyncInfo(
                            on_wait=[w], on_update=[])
                        out.append(n)
                    ins.sync_info = bass_rust.SyncInfo(
                        on_wait=[waits[-1]], on_update=list(si.on_update or []))
                    changed = True
                out.append(ins)
            if changed:
                b.instructions = out


def _prepare(inputs):
    """Host-side folding: per-channel vectors + fused/packed weights."""
    f = lambda k: np.asarray(inputs[k], np.float64)
    alpha = f("alpha").reshape(C)

    s_r = f("bn_rgb_w") / np.sqrt(f("bn_rgb_var") + EPS)
    t_r = f("bn_rgb_b") - f("bn_rgb_mean") * s_r
    s_d = f("bn_depth_w") / np.sqrt(f("bn_depth_var") + EPS)
    t_d = f("bn_depth_b") - f("bn_depth_mean") * s_d

    w_r = np.asarray(inputs["bn_rgb_w"], np.float32)
    w_d = np.asarray(inputs["bn_depth_w"], np.float32)
    idx_r = np.argsort(np.abs(w_r), kind="stable")[:K_EX]
    idx_d = np.argsort(np.abs(w_d), kind="stable")[:K_EX]
    mask_r = np.zeros(C, bool)
    mask_r[idx_r] = True
    mask_d = np.zeros(C, bool)
    mask_d[idx_d] = True

    A1 = np.where(mask_r, alpha * s_r, s_r)
    A2 = np.where(mask_r, (1 - alpha) * s_d, 0.0)
    A3 = np.where(mask_r, alpha * t_r + (1 - alpha) * t_d, t_r)
    D1 = np.where(mask_d, alpha * s_d, s_d)
    D2 = np.where(mask_d, (1 - alpha) * s_r, 0.0)
    D3 = np.where(mask_d, alpha * t_d + (1 - alpha) * t_r, t_d)

    qkv_w = f("qkv_w")
    Wv = qkv_w[2 * C:, :]
    Wc = f("proj_w") @ Wv
    w1, b1 = f("norm1_w"), f("norm1_b")
    Wc_f = Wc * w1[None, :]
    pb = f("proj_b") + Wc @ b1
    wc_rowsum = Wc_f.sum(axis=1)

    w2, b2 = f("norm2_w"), f("norm2_b")
    fc1_f = f("fc1_w") * w2[None, :]
    fb1 = f("fc1_b") + f("fc1_w") @ b2
    fc2_w = f("fc2_w")
    fc2_b = f("fc2_b")
    assert np.allclose(fc2_b, 0.0), "kernel folds fc2_b==0 into V_SCL slot"
    wfh = 0.5 * f("normf_w")
    bf_ = 0.5 * 2.0 * f("normf_b")  # both streams add bf/2; vec holds bf/2
    bf_half = 0.5 * f("normf_b")

    bf16 = ml_dtypes.bfloat16
    fp8 = ml_dtypes.float8_e4m3

    def pack_lhsT(wT, kt, m):
        # wT: [kt*128, m] -> [128, kt*m] with [p, k*m + col] = wT[128k+p, col]
        return np.ascontiguousarray(
            wT.reshape(kt, 128, m).transpose(1, 0, 2).reshape(128, kt * m))

    def pack_lhsT_pairs(wT, kp, m):
        # wT: [kp*256, m] -> [128, kp*2*m], [p, ((q*2+i)*m)+col] =
        #   wT[(2q+i)*128+p, col]   (DoubleRow k-pair layout)
        return np.ascontiguousarray(
            wT.reshape(kp, 2, 128, m).transpose(2, 0, 1, 3).reshape(
                128, kp * 2 * m))

    wc_pack = pack_lhsT(np.ascontiguousarray(Wc_f.T), CT, C).astype(bf16)
    fc1_pack = pack_lhsT_pairs(
        np.ascontiguousarray(fc1_f.T) * WSCALE, CP, MLP).astype(fp8)
    fc2_pack = pack_lhsT_pairs(
        np.ascontiguousarray(fc2_w.T) * WSCALE, MP, C).astype(fp8)

    scl = np.full(C, 1.0 / WSCALE)
    # the ua op computes wfh*(sum - bc_mrs) + bf where the "+bf" is applied
    # once; reference applies mean of (ln*w + b) over 2 tokens = ... + b.
    vv = [A1, A2, A3, D1, D2, D3, pb, scl, wfh, f("normf_b")]
    vecs = np.stack(vv, axis=-1).astype(np.float32)          # [C, NV]
    vecs = vecs.reshape(CT, 128, NV).transpose(1, 0, 2).reshape(128, CT * NV)
    vecs = np.ascontiguousarray(vecs)
    fb1_pack = np.ascontiguousarray(
        fb1.astype(np.float32).reshape(MT, 128).T)           # [128, MT]

    return {
        "wc": wc_pack,
        "fc1": fc1_pack,
        "fc2": fc2_pack,
        "vecs": vecs,
        "fb1": fb1_pack,
        "wcsum": (-wc_rowsum / np.sqrt(C)).astype(bf16).reshape(1, C),
        "ident": np.eye(128, dtype=np.float16),
    }


def _get_runner():
    """Build the Bass module once and cache a jitted shard_map executor."""
    if "runner" in _CACHE:
        return _CACHE["runner"]
    import jax
    from jax.sharding import Mesh, PartitionSpec
    from jax.experimental.shard_map import shard_map
    from concourse import bass2jax

    nc = _build_nc()
    bass2jax.install_neuronx_cc_hook()
    partition_name = (nc.partition_id_tensor.name
                      if nc.partition_id_tensor else None)
    in_names, out_names, out_avals = [], [], []
    for alloc in nc.m.functions[0].allocations:
        if not isinstance(alloc, mybir.MemoryLocationSet):
            continue
        name = alloc.memorylocations[0].name
        if alloc.kind == "ExternalInput":
            if name != partition_name:
                in_names.append(name)
        elif alloc.kind == "ExternalOutput":
            out_names.append(name)
            out_avals.append(jax.core.ShapedArray(
                tuple(alloc.tensor_shape), mybir.dt.np(alloc.dtype)))
    all_in_names = list(in_names) + list(out_names)
    if partition_name is not None:
        all_in_names.append(partition_name)

    def _body(*args):
        operands = list(args)
        if partition_name is not None:
            operands.append(bass2jax.partition_id_tensor())
        return tuple(bass2jax._bass_exec_p.bind(
            *operands, out_avals=tuple(out_avals),
            in_names=tuple(all_in_names), out_names=tuple(out_names),
            lowering_input_output_aliases=(),
            sim_require_finite=True, sim_require_nnan=True, nc=nc))

    devices = jax.devices()[:N_CORES]
    mesh = Mesh(np.asarray(devices), ("core",))
    sharded_args = {"rgb", "dep"}
    in_specs = tuple(
        PartitionSpec("core") if n in sharded_args else PartitionSpec()
        for n in in_names) + (PartitionSpec("core"),) * len(out_names)
    fn = jax.jit(
        shard_map(_body, mesh=mesh,
                  in_specs=in_specs,
                  out_specs=(PartitionSpec("core"),) * len(out_names),
                  check_rep=False),
        keep_unused=True)
    zeros = [jax.device_put(
        np.zeros((a.shape[0] * N_CORES,) + tuple(a.shape[1:]), a.dtype))
        for a in out_avals]
    _CACHE["runner"] = (fn, in_names, zeros, jax)
    return _CACHE["runner"]


def kernel(**inputs) -> np.ndarray:
    rgb = np.asarray(inputs["rgb"], np.float32).astype(np.float16)
    dep = np.asarray(inputs["depth"], np.float32).astype(np.float16)
    consts = _prepare(inputs)

    fn, in_names, zeros, jax = _get_runner()
    vals = {
        "rgb": np.ascontiguousarray(rgb.reshape(ROWS * N_CORES, C)),
        "dep": np.ascontiguousarray(dep.reshape(ROWS * N_CORES, C)),
    }
    vals.update(consts)
    args = [vals[n] for n in in_names] + list(zeros)
    outs = fn(*args)
    out = np.asarray(outs[0]).reshape(B, T, C)
    return out


if __name__ == "__main__":
    print("built module ok" if _build_nc() else "")


# revision 14
# speedup vs baseline: 1.9824x; 1.0366x over previous
"""Trainium2 Bass kernel for nn_CMFuser (topk_masking) — v2.

Self-contained: accepts FULL inputs (as produced by setup_inputs()), returns
the FULL [32, 512, 768] output. Internally shards batch across 8 NeuronCores
(pure data parallel, 4 batches/core) and runs a hand-written Bass/Tile kernel.

Algorithmic structure (same folding as v1, validated vs the jax reference):
  * BN(eval) + topk-channel-exchange blend folds into per-channel affine:
        x0_rgb = A1*rgb + A2*depth + A3,   x0_depth = D1*depth + D2*rgb + D3
  * The 2-token attention with -1e9 diag mask is an EXACT token swap, so
    qkv+softmax+proj collapse into one fused C x C matmul Wc = proj_w @ Wv
    applied to the OTHER token.
  * LN weights fold into the following matmul; LN mean-subtraction folds
    into a rank-1 (K=1) matmul correction (norm1) / bcast subtract (norm2).
  * Final LN + mean over the 2 modality tokens folds into 0.5*wf scale.

v2 performance changes vs the 895us baseline:
  * fc1/fc2 matmuls in fp8e4m3 with DoubleRow perf mode (0.5 cyc/row and
    K=256 per instruction -> 4x the bf16 PE throughput). Weights are
    pre-scaled by 16 before quantization to dodge the e4m3 subnormal range;
    the 1/16 descale folds into the GELU input scale and the fc2-residual
    per-channel multiplier. Wc stays bf16 (fp8 there pushes rel-err > 2e-2).
  * Whole residual stream, inputs, transposes and LN stats in fp16:
    - input DMA traffic halved (host casts rgb/depth to fp16),
    - PE transposes at 1 cyc/row (fp16 identity),
    - DVE elementwise ops at 2x (2-byte dtypes).
  * Elementwise work spread across DVE / ACT / Pool by measured budget.
Measured error of this exact quantization recipe vs the reference
(numpy emulation): 1.45e-2 < 2e-2.
"""

import os
import sys

sys.path.insert(0, "/opt/trn_rl_repo")

import numpy as np
import ml_dtypes

import concourse.bass as bass
import concourse.mybir as mybir
import concourse.tile as tile
from contextlib import ExitStack

dt = mybir.dt
Alu = mybir.AluOpType
Act = mybir.ActivationFunctionType
PerfMode = mybir.MatmulPerfMode

B, T, C = 32, 512, 768
H = 4
K_EX = int(C * 0.2)
MLP = 4 * C
EPS = 1e-5
N_CORES = 8
B_CORE = B // N_CORES          # 4 batches per core
ROWS = B_CORE * T              # 2048 token-sites per core
TG = 512                       # tokens per group
NG = ROWS // TG                # 4 groups per core
CT = C // 128                  # 6 channel tiles
CP = CT // 2                   # 3 channel k-pairs (DoubleRow)
MT = MLP // 128                # 24 mlp tiles
MP = MT // 2                   # 12 mlp k-pairs
NTT = TG // 128                # 4 token tiles per group
WSCALE = 16.0                  # fp8 weight pre-scale (descaled on device)

# vector slot indices in the packed per-channel constant table
V_A1, V_A2, V_A3, V_D1, V_D2, V_D3, V_PB, V_SCL, V_WFH, V_BF = range(10)
NV = 10

_CACHE = {}


def _build_nc(legalize=True):
    """Build the per-core Bass module (same program on all 8 cores)."""
    nc = bass.Bass()

    rgb_d = nc.dram_tensor("rgb", [ROWS, C], dt.float16, kind="ExternalInput")
    dep_d = nc.dram_tensor("dep", [ROWS, C], dt.float16, kind="ExternalInput")
    wc_d = nc.dram_tensor("wc", [128, CT * C], dt.bfloat16, kind="ExternalInput")
    fc1_d = nc.dram_tensor("fc1", [128, CP * 2 * MLP], dt.float8e4,
                           kind="ExternalInput")
    fc2_d = nc.dram_tensor("fc2", [128, MP * 2 * C], dt.float8e4,
                           kind="ExternalInput")
    vecs_d = nc.dram_tensor("vecs", [128, CT * NV], dt.float32,
                            kind="ExternalInput")
    fb1_d = nc.dram_tensor("fb1", [128, MT], dt.float32, kind="ExternalInput")
    wcsum_d = nc.dram_tensor("wcsum", [1, C], dt.bfloat16, kind="ExternalInput")
    ident_d = nc.dram_tensor("ident", [128, 128], dt.float16,
                             kind="ExternalInput")
    out_d = nc.dram_tensor("out", [ROWS, C], dt.float32, kind="ExternalOutput")

    with tile.TileContext(nc) as tc, ExitStack() as ctx:
        const = ctx.enter_context(tc.tile_pool(name="const", bufs=1))
        inp = ctx.enter_context(tc.tile_pool(name="inp", bufs=16))
        xp = ctx.enter_context(tc.tile_pool(name="xp", bufs=26))
        sqp = ctx.enter_context(tc.tile_pool(name="sqp", bufs=8))
        h1p = ctx.enter_context(tc.tile_pool(name="h1p", bufs=14))
        h2p = ctx.enter_context(tc.tile_pool(name="h2p", bufs=8))
        a8p = ctx.enter_context(tc.tile_pool(name="a8p", bufs=15))
        bcp = ctx.enter_context(tc.tile_pool(name="bcp", bufs=8))
        tmpp = ctx.enter_context(tc.tile_pool(name="tmpp", bufs=6))
        rows = ctx.enter_context(tc.tile_pool(name="rows", bufs=10))
        rows1 = ctx.enter_context(tc.tile_pool(name="rows1", bufs=8))
        uap = ctx.enter_context(tc.tile_pool(name="uap", bufs=8))
        outp = ctx.enter_context(tc.tile_pool(name="outp", bufs=3))
        psum = ctx.enter_context(tc.tile_pool(name="psum", bufs=2, space="PSUM"))

        # ---- constants / weights (small, early-needed tensors first) ----
        ident_sb = const.tile([128, 128], dt.float16)
        nc.sync.dma_start(ident_sb[:], ident_d[:])
        vecs_sb = const.tile([128, CT * NV], dt.float32)
        nc.sync.dma_start(vecs_sb[:], vecs_d[:])
        fb1_sb = const.tile([128, MT], dt.float32)
        nc.sync.dma_start(fb1_sb[:], fb1_d[:])
        wcsum_sb = const.tile([1, C], dt.bfloat16)
        nc.sync.dma_start(wcsum_sb[:], wcsum_d[:])
        # group-0 inputs issued BEFORE the big weight loads
        pre_in = {}
        for s_, src_ in ((0, rgb_d), (1, dep_d)):
            for tt_ in range(NTT):
                it_ = inp.tile([128, C], dt.float16, tag="in",
                               name=f"in_0_{s_}_{tt_}")
                nc.sync.dma_start(it_[:], src_[tt_ * 128:(tt_ + 1) * 128, :])
                pre_in[s_, tt_] = it_
        wc_sb = const.tile([128, CT * C], dt.bfloat16)
        nc.sync.dma_start(wc_sb[:], wc_d[:])
        fc1_sb = const.tile([128, CP, 2, MLP], dt.float8e4)
        nc.sync.dma_start(fc1_sb[:, :, :, :], fc1_d[:, :])
        fc2_sb = const.tile([128, MP, 2, C], dt.float8e4)
        nc.sync.dma_start(fc2_sb[:, :, :, :], fc2_d[:, :])

        ones16 = const.tile([128, 1], dt.float16)
        nc.vector.memset(ones16[:], 1.0)
        sqrtc_f32 = const.tile([1, 128], dt.float32)
        nc.vector.memset(sqrtc_f32[:], float(np.sqrt(C)))
        sqrtc_row = const.tile([1, 128], dt.float32r)
        with nc.allow_low_precision("fp32r bcast lhsT"):
            nc.vector.tensor_copy(sqrtc_row[:], sqrtc_f32[:])
        isqrtc_row_b = const.tile([1, 128], dt.bfloat16)
        nc.vector.memset(isqrtc_row_b[:], float(1.0 / np.sqrt(C)))
        ceps_ap = const.tile([1, 1], dt.float32)
        nc.vector.memset(ceps_ap[:], float(C * EPS))

        def vec(idx, j):
            return vecs_sb[:, j * NV + idx: j * NV + idx + 1]

        # per-group state dicts
        in_tiles = [None] * NG
        x = [None] * NG            # (s, j) -> [128,TG] fp16 residual tiles
        st_rows = [None] * NG      # per-LN row results
        h1 = [None] * NG
        h2pair = [None] * NG

        def stage_L(g):
            """input DMA (g>0), PE transpose, DVE blend -> x0."""
            if g == 0:
                in_tiles[g] = pre_in
            else:
                it_map = {}
                r0 = g * TG
                for s, src in ((0, rgb_d), (1, dep_d)):
                    for tt_ in range(NTT):
                        it = inp.tile([128, C], dt.float16, tag="in",
                                      name=f"in_{g}_{s}_{tt_}")
                        nc.sync.dma_start(
                            it[:], src[r0 + tt_ * 128: r0 + (tt_ + 1) * 128, :])
                        it_map[s, tt_] = it
                in_tiles[g] = it_map
            xg = {}
            for j in range(CT):
                pt = {}
                for s in (0, 1):
                    p = psum.tile([128, TG], dt.float16, tag="tp", bufs=3,
                                  padded_shape=[128, 1024],
                                  name=f"pt_{g}_{s}_{j}")
                    for tt_ in range(NTT):
                        nc.tensor.transpose(
                            p[:, tt_ * 128:(tt_ + 1) * 128],
                            in_tiles[g][s, tt_][:, j * 128:(j + 1) * 128],
                            ident_sb[:])
                    pt[s] = p
                t1 = tmpp.tile([128, TG], dt.float16, tag="bl",
                               name=f"t1_{g}_{j}")
                nc.vector.tensor_scalar(t1[:], pt[1][:], vec(V_A2, j),
                                        vec(V_A3, j), Alu.mult, Alu.add)
                x0r = xp.tile([128, TG], dt.float16, tag="res",
                              name=f"x0r_{g}_{j}")
                nc.vector.scalar_tensor_tensor(x0r[:], pt[0][:], vec(V_A1, j),
                                               t1[:], Alu.mult, Alu.add)
                t2 = tmpp.tile([128, TG], dt.float16, tag="bl",
                               name=f"t2_{g}_{j}")
                nc.vector.tensor_scalar(t2[:], pt[0][:], vec(V_D2, j),
                                        vec(V_D3, j), Alu.mult, Alu.add)
                x0d = xp.tile([128, TG], dt.float16, tag="res",
                              name=f"x0d_{g}_{j}")
                nc.vector.scalar_tensor_tensor(x0d[:], pt[1][:], vec(V_D1, j),
                                               t2[:], Alu.mult, Alu.add)
                xg[0, j] = x0r
                xg[1, j] = x0d
            x[g] = xg

        def ln_stats(g, name, sq_engine="pool"):
            """LN stats over channels for both streams of group g.

            Returns {('r'|'d'): (rrow_f32r, mr_bf16)}; rows are [1,TG].
            PE: 12 fp16 matmuls per stream into one psum stat tile
            (sum at partition 0, sum-of-squares at partition 32).
            """
            out = {}
            for s in (0, 1):
                sfx = "r" if s == 0 else "d"
                sq = []
                for j in range(CT):
                    sqt = sqp.tile([128, TG], dt.float16, tag="sq",
                                   name=f"sq_{name}_{s}_{j}")
                    if sq_engine == "vector":
                        nc.vector.tensor_tensor(sqt[:], x[g][s, j][:],
                                                x[g][s, j][:], Alu.mult)
                    elif sq_engine == "pool":
                        nc.gpsimd.tensor_tensor(sqt[:], x[g][s, j][:],
                                                x[g][s, j][:], Alu.mult)
                    else:
                        nc.scalar.square(sqt[:], x[g][s, j][:])
                    sq.append(sqt)
                stat = psum.tile([128, TG], dt.float32, tag="ps",
                                 name=f"stat_{name}_{s}")
                for j in range(CT):
                    nc.tensor.matmul(stat[0:1, :], ones16[:], x[g][s, j][:],
                                     tile_position=(0, 0),
                                     start=(j == 0), stop=(j == CT - 1))
                    nc.tensor.matmul(stat[32:33, :], ones16[:], sq[j][:],
                                     tile_position=(0, 32),
                                     start=(j == 0), stop=(j == CT - 1))
                sq1 = rows.tile([1, TG], dt.float32, tag="rows",
                                name=f"sq1_{name}_{s}")
                nc.scalar.square(sq1[:], stat[0:1, :])
                u = rows.tile([1, TG], dt.float32, tag="rows",
                              name=f"u_{name}_{s}")
                nc.vector.scalar_tensor_tensor(u[:], sq1[:], -1.0 / C,
                                               stat[32:33, :], Alu.mult,
                                               Alu.add)
                std = rows.tile([1, TG], dt.float32, tag="rows",
                                name=f"std_{name}_{s}")
                nc.scalar.activation(std[:], u[:], Act.Sqrt,
                                     bias=ceps_ap[0:1, 0:1], scale=1.0)
                rrow = rows1.tile([1, TG], dt.float32r, tag="rows1", bufs=4,
                                  name=f"r_{name}_{s}")
                with nc.allow_low_precision("fp32r bcast rows"):
                    nc.vector.reciprocal(rrow[:], std[:])
                mr = rows1.tile([1, TG], dt.bfloat16, tag="rows1b", bufs=4,
                                name=f"mr_{name}_{s}")
                nc.vector.tensor_tensor(mr[:], stat[0:1, :], rrow[:], Alu.mult)
                out[sfx] = (rrow, mr)
            return out

        def bcast_r16(rrow, name):
            """rsqrt row broadcast: K=1 PE matmul + DVE copy to fp16 SBUF."""
            bc = psum.tile([128, TG], dt.float32, tag="ps", name=f"bc_{name}")
            nc.tensor.matmul(bc[:], sqrtc_row[0:1, :], rrow[:],
                             start=True, stop=True)
            bc16 = bcp.tile([128, TG], dt.float16, tag="bc16",
                            name=f"bc16_{name}")
            nc.vector.tensor_copy(bc16[:], bc[:])
            return bc16

        def stage_W(g):
            """norm1 apply + Wc (attention swap) + residual -> x1."""
            st = st_rows[g]
            h1g = {}
            for s in (0, 1):
                sfx = "r" if s == 0 else "d"
                bc16 = bcast_r16(st[sfx][0], f"n1_{g}_{s}")
                for j in range(CT):
                    ht = h1p.tile([128, TG], dt.bfloat16, tag="h1",
                                  name=f"h1_{g}_{s}_{j}")
                    nc.gpsimd.tensor_tensor(ht[:], x[g][s, j][:], bc16[:],
                                            Alu.mult)
                    h1g[s, j] = ht
            h1[g] = h1g
            for s, o in ((0, 1), (1, 0)):
                mr = st["r" if s == 0 else "d"][1]
                # mo-halves of 3 so only 3 "acc" psum banks are held at once
                for half in (0, 1):
                    mos = range(3 * half, 3 * half + 3)
                    accs = {}
                    for mo in mos:
                        accs[mo] = psum.tile([128, TG], dt.float32, tag="acc",
                                             bufs=3, name=f"g_{g}_{s}_{mo}")
                    for k in range(CT):
                        for mo in mos:
                            nc.tensor.matmul(
                                accs[mo][:],
                                wc_sb[:,
                                      k * C + mo * 128: k * C + (mo + 1) * 128],
                                h1g[s, k][:], start=(k == 0), stop=False)
                    for mo in mos:
                        nc.tensor.matmul(
                            accs[mo][:],
                            wcsum_sb[0:1, mo * 128:(mo + 1) * 128],
                            mr[:], start=False, stop=True)
                        # x1_o = x0_o + g_s + pb (in place), o = other stream
                        # (gpsimd cannot read PSUM -> DVE)
                        nc.vector.scalar_tensor_tensor(x[g][o, mo][:],
                                                       accs[mo][:],
                                                       vec(V_PB, mo),
                                                       x[g][o, mo][:],
                                                       Alu.add, Alu.add)

        def stage_M(g):
            """norm2 apply + fp8 DoubleRow MLP + residual -> x2."""
            st = st_rows[g]
            h2g = {}
            for s in (0, 1):
                sfx = "r" if s == 0 else "d"
                rrow, mr = st[sfx]
                bc16 = bcast_r16(rrow, f"n2_{g}_{s}")
                bcm = psum.tile([128, TG], dt.float32, tag="ps",
                                name=f"bcm_{g}_{s}")
                nc.tensor.matmul(bcm[:], isqrtc_row_b[0:1, :], mr[:],
                                 start=True, stop=True)
                bcm16 = bcp.tile([128, TG], dt.float16, tag="bc16",
                                 name=f"bcm16_{g}_{s}")
                nc.vector.tensor_copy(bcm16[:], bcm[:])
                for kp in range(CP):
                    pair = h2p.tile([128, 2, TG], dt.float8e4, tag="h2",
                                    name=f"h2_{g}_{s}_{kp}")
                    for i in (0, 1):
                        j = 2 * kp + i
                        t_ = tmpp.tile([128, TG], dt.float16, tag="bl",
                                       name=f"h2t_{g}_{s}_{j}")
                        nc.gpsimd.tensor_tensor(t_[:], x[g][s, j][:], bc16[:],
                                                Alu.mult)
                        nc.gpsimd.tensor_tensor(pair[:, i, :], t_[:],
                                                bcm16[:], Alu.subtract)
                    h2g[s, kp] = pair
            for s in (0, 1):
                # fc2 accumulates in co-halves of 3 banks; the second half
                # replays the (persistent) a8 pairs after the m-loop.
                acc2 = {}
                for co in range(3):
                    acc2[co] = psum.tile([128, TG], dt.float32, tag="acc",
                                         bufs=3, name=f"acc_{g}_{s}_{co}")
                apairs = []
                apair = None
                for m in range(MT):
                    pf = psum.tile([128, TG], dt.float32, tag="ps",
                                   name=f"pf_{g}_{s}_{m}")
                    for kp in range(CP):
                        nc.tensor.matmul(
                            pf[:],
                            fc1_sb[:, kp, :, m * 128:(m + 1) * 128],
                            h2g[s, kp][:, :, :],
                            start=(kp == 0), stop=(kp == CP - 1),
                            perf_mode=PerfMode.DoubleRow)
                    if m % 2 == 0:
                        apair = a8p.tile([128, 2, TG], dt.float8e4, tag="a8",
                                         name=f"a_{g}_{s}_{m // 2}")
                        apairs.append(apair)
                    nc.scalar.activation(apair[:, m % 2, :], pf[:], Act.Gelu,
                                         bias=fb1_sb[:, m:m + 1],
                                         scale=float(1.0 / WSCALE))
                    if m % 2 == 1:
                        mp = m // 2
                        for co in range(3):
                            nc.tensor.matmul(
                                acc2[co][:],
                                fc2_sb[:, mp, :, co * 128:(co + 1) * 128],
                                apair[:, :, :],
                                start=(mp == 0), stop=(mp == MP - 1),
                                perf_mode=PerfMode.DoubleRow)
                for co in range(3):
                    # x2 = acc/WSCALE + x1 (fc2_b guaranteed zero; V_SCL=1/16)
                    nc.vector.scalar_tensor_tensor(x[g][s, co][:],
                                                   acc2[co][:],
                                                   vec(V_SCL, co),
                                                   x[g][s, co][:],
                                                   Alu.mult, Alu.add)
                acc2b = {}
                for co in range(3, CT):
                    acc2b[co] = psum.tile([128, TG], dt.float32, tag="acc",
                                          bufs=3, name=f"acc_{g}_{s}_{co}")
                for mp in range(MP):
                    for co in range(3, CT):
                        nc.tensor.matmul(
                            acc2b[co][:],
                            fc2_sb[:, mp, :, co * 128:(co + 1) * 128],
                            apairs[mp][:, :, :],
                            start=(mp == 0), stop=(mp == MP - 1),
                            perf_mode=PerfMode.DoubleRow)
                for co in range(3, CT):
                    nc.vector.scalar_tensor_tensor(x[g][s, co][:],
                                                   acc2b[co][:],
                                                   vec(V_SCL, co),
                                                   x[g][s, co][:],
                                                   Alu.mult, Alu.add)

        def stage_F(g):
            """final norm + modality mean + transpose out + DMA."""
            st = st_rows[g]
            bc_rr16 = bcast_r16(st["r"][0], f"nf_{g}_r")
            bc_rd16 = bcast_r16(st["d"][0], f"nf_{g}_d")
            bc_mrs = psum.tile([128, TG], dt.float32, tag="ps",
                               name=f"bcmrs_{g}")
            nc.tensor.matmul(bc_mrs[:], isqrtc_row_b[0:1, :], st["r"][1][:],
                             start=True, stop=False)
            nc.tensor.matmul(bc_mrs[:], isqrtc_row_b[0:1, :], st["d"][1][:],
                             start=False, stop=True)
            bc_mrs16 = bcp.tile([128, TG], dt.float16, tag="bc16",
                                name=f"bcmrs16_{g}")
            nc.vector.tensor_copy(bc_mrs16[:], bc_mrs[:])
            uas = []
            for j in range(CT):
                s1 = tmpp.tile([128, TG], dt.float16, tag="bl",
                               name=f"nf1_{g}_{j}")
                nc.gpsimd.tensor_tensor(s1[:], x[g][0, j][:], bc_rr16[:],
                                        Alu.mult)
                s2 = tmpp.tile([128, TG], dt.float16, tag="bl",
                               name=f"nf2_{g}_{j}")
                nc.gpsimd.tensor_tensor(s2[:], x[g][1, j][:], bc_rd16[:],
                                        Alu.mult)
                nc.gpsimd.tensor_tensor(s1[:], s1[:], s2[:], Alu.add)
                nc.gpsimd.tensor_tensor(s1[:], s1[:], bc_mrs16[:],
                                        Alu.subtract)
                ua = uap.tile([128, TG], dt.float16, tag="uaff",
                              name=f"ua_{g}_{j}")
                nc.vector.tensor_scalar(ua[:], s1[:], vec(V_WFH, j),
                                        vec(V_BF, j), Alu.mult, Alu.add)
                uas.append(ua)
            r0 = g * TG
            for tt_ in range(NTT):
                po = psum.tile([128, TG], dt.float16, tag="tp", bufs=3,
                               padded_shape=[128, 1024], name=f"po_{g}_{tt_}")
                po2 = psum.tile([128, TG], dt.float16, tag="tp", bufs=3,
                                padded_shape=[128, 1024],
                                name=f"po2_{g}_{tt_}")
                for j in range(CT):
                    dst = (po[:, j * 128:(j + 1) * 128] if j < 4
                           else po2[:, (j - 4) * 128:(j - 3) * 128])
                    nc.tensor.transpose(
                        dst, uas[j][:, tt_ * 128:(tt_ + 1) * 128], ident_sb[:])
                ot = outp.tile([128, C], dt.float32, tag="ot",
                               name=f"ot_{g}_{tt_}")
                nc.scalar.copy(ot[:, 0:512], po[:, :])
                nc.scalar.copy(ot[:, 512:768], po2[:, 0:256])
                nc.sync.dma_start(
                    out_d[r0 + tt_ * 128: r0 + (tt_ + 1) * 128, :], ot[:])

        # ---- software-pipelined schedule over groups ----
        def S1(g):
            st_rows[g] = ln_stats(g, f"n1_{g}", sq_engine="pool")

        def S2(g):
            st_rows[g] = ln_stats(g, f"n2_{g}", sq_engine="pool")

        def SF(g):
            st_rows[g] = ln_stats(g, f"nf_{g}", sq_engine="pool")

        # software pipeline: next group's load/stats fill this group's
        # PE dependency gaps (esp. around the MLP and Wc phases).
        sched = [(stage_L, 0), (S1, 0), (stage_W, 0), (S2, 0)]
        for g in range(NG):
            if g + 1 < NG:
                sched += [(stage_L, g + 1), (stage_M, g), (S1, g + 1),
                          (SF, g), (stage_W, g + 1), (stage_F, g),
                          (S2, g + 1)]
            else:
                sched += [(stage_M, g), (SF, g), (stage_F, g)]
        for fn, g in sched:
            fn(g)

    if legalize:
        _legalize_waits(nc)
    nc.finalize()
    return nc


def _legalize_waits(nc):
    """Walrus ISA structs have at most 1-2 sync-wait slots per instruction,
    but Tile's wait assignment can emit more. Move excess waits onto
    same-engine NoOps inserted immediately before the offending
    instruction."""
    import bass_rust
    nop_i = [0]
    for f in nc.m.functions:
        for b in f.blocks:
            insts = b.instructions
            out = []
            changed = False
            for ins in insts:
                si = getattr(ins, "sync_info", None)
                waits = list(si.on_wait) if (si and si.on_wait) else []
                if len(waits) > 1:
                    eng = ins.engine
                    for w in waits[:-1]:
                        n = bass_rust.InstNoOp(name=f"I-nopw-{nop_i[0]}")
                        nop_i[0] += 1
                        n.engine = eng
                        n.sync_info = bass_rust.SyncInfo(
                            on_wait=[w], on_update=[])
                        out.append(n)
                    ins.sync_info = bass_rust.SyncInfo(
                        on_wait=[waits[-1]], on_update=list(si.on_update or []))
                    changed = True
                out.append(ins)
            if changed:
                b.instructions = out


def _prepare(inputs):
    """Host-side folding: per-channel vectors + fused/packed weights."""
    f = lambda k: np.asarray(inputs[k], np.float64)
    alpha = f("alpha").reshape(C)

    s_r = f("bn_rgb_w") / np.sqrt(f("bn_rgb_var") + EPS)
    t_r = f("bn_rgb_b") - f("bn_rgb_mean") * s_r
    s_d = f("bn_depth_w") / np.sqrt(f("bn_depth_var") + EPS)
    t_d = f("bn_depth_b") - f("bn_depth_mean") * s_d

    w_r = np.asarray(inputs["bn_rgb_w"], np.float32)
    w_d = np.asarray(inputs["bn_depth_w"], np.float32)
    idx_r = np.argsort(np.abs(w_r), kind="stable")[:K_EX]
    idx_d = np.argsort(np.abs(w_d), kind="stable")[:K_EX]
    mask_r = np.zeros(C, bool)
    mask_r[idx_r] = True
    mask_d = np.zeros(C, bool)
    mask_d[idx_d] = True

    A1 = np.where(mask_r, alpha * s_r, s_r)
    A2 = np.where(mask_r, (1 - alpha) * s_d, 0.0)
    A3 = np.where(mask_r, alpha * t_r + (1 - alpha) * t_d, t_r)
    D1 = np.where(mask_d, alpha * s_d, s_d)
    D2 = np.where(mask_d, (1 - alpha) * s_r, 0.0)
    D3 = np.where(mask_d, alpha * t_d + (1 - alpha) * t_r, t_d)

    qkv_w = f("qkv_w")
    Wv = qkv_w[2 * C:, :]
    Wc = f("proj_w") @ Wv
    w1, b1 = f("norm1_w"), f("norm1_b")
    Wc_f = Wc * w1[None, :]
    pb = f("proj_b") + Wc @ b1
    wc_rowsum = Wc_f.sum(axis=1)

    w2, b2 = f("norm2_w"), f("norm2_b")
    fc1_f = f("fc1_w") * w2[None, :]
    fb1 = f("fc1_b") + f("fc1_w") @ b2
    fc2_w = f("fc2_w")
    fc2_b = f("fc2_b")
    assert np.allclose(fc2_b, 0.0), "kernel folds fc2_b==0 into V_SCL slot"
    wfh = 0.5 * f("normf_w")
    bf_ = 0.5 * 2.0 * f("normf_b")  # both streams add bf/2; vec holds bf/2
    bf_half = 0.5 * f("normf_b")

    bf16 = ml_dtypes.bfloat16
    fp8 = ml_dtypes.float8_e4m3

    def pack_lhsT(wT, kt, m):
        # wT: [kt*128, m] -> [128, kt*m] with [p, k*m + col] = wT[128k+p, col]
        return np.ascontiguousarray(
            wT.reshape(kt, 128, m).transpose(1, 0, 2).reshape(128, kt * m))

    def pack_lhsT_pairs(wT, kp, m):
        # wT: [kp*256, m] -> [128, kp*2*m], [p, ((q*2+i)*m)+col] =
        #   wT[(2q+i)*128+p, col]   (DoubleRow k-pair layout)
        return np.ascontiguousarray(
            wT.reshape(kp, 2, 128, m).transpose(2, 0, 1, 3).reshape(
                128, kp * 2 * m))

    wc_pack = pack_lhsT(np.ascontiguousarray(Wc_f.T), CT, C).astype(bf16)
    fc1_pack = pack_lhsT_pairs(
        np.ascontiguousarray(fc1_f.T) * WSCALE, CP, MLP).astype(fp8)
    fc2_pack = pack_lhsT_pairs(
        np.ascontiguousarray(fc2_w.T) * WSCALE, MP, C).astype(fp8)

    scl = np.full(C, 1.0 / WSCALE)
    # the ua op computes wfh*(sum - bc_mrs) + bf where the "+bf" is applied
    # once; reference applies mean of (ln*w + b) over 2 tokens = ... + b.
    vv = [A1, A2, A3, D1, D2, D3, pb, scl, wfh, f("normf_b")]
    vecs = np.stack(vv, axis=-1).astype(np.float32)          # [C, NV]
    vecs = vecs.reshape(CT, 128, NV).transpose(1, 0, 2).reshape(128, CT * NV)
    vecs = np.ascontiguousarray(vecs)
    fb1_pack = np.ascontiguousarray(
        fb1.astype(np.float32).reshape(MT, 128).T)           # [128, MT]

    return {
        "wc": wc_pack,
        "fc1": fc1_pack,
        "fc2": fc2_pack,
        "vecs": vecs,
        "fb1": fb1_pack,
        "wcsum": (-wc_rowsum / np.sqrt(C)).astype(bf16).reshape(1, C),
        "ident": np.eye(128, dtype=np.float16),
    }


def _get_runner():
    """Build the Bass module once and cache a jitted shard_map executor."""
    if "runner" in _CACHE:
        return _CACHE["runner"]
    import jax
    from jax.sharding import Mesh, PartitionSpec
    from jax.experimental.shard_map import shard_map
    from concourse import bass2jax

    nc = _build_nc()
    bass2jax.install_neuronx_cc_hook()
    partition_name = (nc.partition_id_tensor.name
                      if nc.partition_id_tensor else None)
    in_names, out_names, out_avals = [], [], []
    for alloc in nc.m.functions[0].allocations:
        if not isinstance(alloc, mybir.MemoryLocationSet):
            continue
        name = alloc.memorylocations[0].name
        if alloc.kind == "ExternalInput":
            if name != partition_name:
                in_names.append(name)
        elif alloc.kind == "ExternalOutput":
            out_names.append(name)
            out_avals.append(jax.core.ShapedArray(
                tuple(alloc.tensor_shape), mybir.dt.np(alloc.dtype)))
    all_in_names = list(in_names) + list(out_names)
    if partition_name is not None:
        all_in_names.append(partition_name)

    def _body(*args):
        operands = list(args)
        if partition_name is not None:
            operands.append(bass2jax.partition_id_tensor())
        return tuple(bass2jax._bass_exec_p.bind(
            *operands, out_avals=tuple(out_avals),
            in_names=tuple(all_in_names), out_names=tuple(out_names),
            lowering_input_output_aliases=(),
            sim_require_finite=True, sim_require_nnan=True, nc=nc))

    devices = jax.devices()[:N_CORES]
    mesh = Mesh(np.asarray(devices), ("core",))
    sharded_args = {"rgb", "dep"}
    in_specs = tuple(
        PartitionSpec("core") if n in sharded_args else PartitionSpec()
        for n in in_names) + (PartitionSpec("core"),) * len(out_names)
    fn = jax.jit(
        shard_map(_body, mesh=mesh,
                  in_specs=in_specs,
                  out_specs=(PartitionSpec("core"),) * len(out_names),
                  check_rep=False),
        keep_unused=True)
    zeros = [jax.device_put(
        np.zeros((a.shape[0] * N_CORES,) + tuple(a.shape[1:]), a.dtype))
        for a in out_avals]
    _CACHE["runner"] = (fn, in_names, zeros, jax)
    return _CACHE["runner"]


def kernel(**inputs) -> np.ndarray:
    rgb = np.asarray(inputs["rgb"], np.float32).astype(np.float16)
    dep = np.asarray(inputs["depth"], np.float32).astype(np.float16)
    consts = _prepare(inputs)

    fn, in_names, zeros, jax = _get_runner()
    vals = {
        "rgb": np.ascontiguousarray(rgb.reshape(ROWS * N_CORES, C)),
        "dep": np.ascontiguousarray(dep.reshape(ROWS * N_CORES, C)),
    }
    vals.update(consts)
    args = [vals[n] for n in in_names] + list(zeros)
    outs = fn(*args)
    out = np.asarray(outs[0]).reshape(B, T, C)
    return out


if __name__ == "__main__":
    print("built module ok" if _build_nc() else "")


# revision 18
# speedup vs baseline: 2.1498x; 1.0844x over previous
"""Trainium2 Bass kernel for nn_CMFuser (topk_masking) — v2.2.

Self-contained: accepts FULL inputs (as produced by setup_inputs()), returns
the FULL [32, 512, 768] output. Internally shards batch across 8 NeuronCores
(pure data parallel, 4 batches/core) and runs a hand-written Bass/Tile kernel.

Algorithmic structure (validated against the jax reference):
  * BN(eval) + topk-channel-exchange blend folds into per-channel affine:
        x0_rgb = A1*rgb + A2*depth + A3,   x0_depth = D1*depth + D2*rgb + D3
  * The 2-token attention with -1e9 diag mask is an EXACT token swap, so
    qkv+softmax+proj collapse into one fused C x C matmul Wc = proj_w @ Wv
    applied to the OTHER token.
  * LN weights fold into the following matmul; LN mean-subtraction folds
    into a rank-1 (K=1) matmul correction (norm1) / bcast subtract (norm2).
  * Final LN + mean over the 2 modality tokens folds into 0.5*wf scale.

Performance structure (vs the 895us bf16 baseline):
  * fc1/fc2 in fp8e4m3 DoubleRow (0.5 cyc/row, K=256/instr = 4x bf16 PE
    throughput). Weights pre-scaled x16 before quantization to dodge the
    e4m3 subnormal range; descale folds into the GELU input scale and the
    residual-add per-channel multiplier.
  * Wc in COMPENSATED fp8 DoubleRow: h1 is split into hi = fp8(h1) and
    lo = fp8(h1 - hi); Wc@(hi+lo) recovers ~bf16 accuracy at 2x bf16 speed.
    pb (== 0 for this model) folds away; the x16 weight scale descales in
    the residual add.
  * Whole residual stream, inputs, transposes and LN stats in fp16
    (half the DMA, 1 cyc/row transposes, 2-byte DVE ops).
  * MLP m-loop interleaves the two modality streams so ACT (GELU) and PE
    (fp8 matmuls) are both ~saturated; fc2 output-columns 1..5 are swept
    from the persistent fp8 activation pairs after the loop (dense PE).
  * Software-pipelined group schedule; all input DMAs prefetched upfront.
Measured error of this quantization recipe vs the reference: 1.49e-2 < 2e-2.
"""

import os
import sys

sys.path.insert(0, "/opt/trn_rl_repo")

import numpy as np
import ml_dtypes

import concourse.bass as bass
import concourse.mybir as mybir
import concourse.tile as tile
from contextlib import ExitStack

dt = mybir.dt
Alu = mybir.AluOpType
Act = mybir.ActivationFunctionType
PerfMode = mybir.MatmulPerfMode

B, T, C = 32, 512, 768
H = 4
K_EX = int(C * 0.2)
MLP = 4 * C
EPS = 1e-5
N_CORES = 8
B_CORE = B // N_CORES          # 4 batches per core
ROWS = B_CORE * T              # 2048 token-sites per core
TG = 512                       # tokens per group
NG = ROWS // TG                # 4 groups per core
CT = C // 128                  # 6 channel tiles
CP = CT // 2                   # 3 channel k-pairs (DoubleRow)
MT = MLP // 128                # 24 mlp tiles
MP = MT // 2                   # 12 mlp k-pairs
NTT = TG // 128                # 4 token tiles per group
WSCALE = 16.0                  # fp8 weight pre-scale (descaled on device)

# vector slot indices in the packed per-channel constant table
V_A1, V_A2, V_A3, V_D1, V_D2, V_D3, V_SCLW, V_SCL, V_WFH, V_BF = range(10)
NV = 10

_CACHE = {}


def _build_nc(legalize=True):
    """Build the per-core Bass module (same program on all 8 cores)."""
    nc = bass.Bass()

    rgb_d = nc.dram_tensor("rgb", [ROWS, C], dt.float16, kind="ExternalInput")
    dep_d = nc.dram_tensor("dep", [ROWS, C], dt.float16, kind="ExternalInput")
    wc_d = nc.dram_tensor("wc", [128, CP * 2 * C], dt.float8e4,
                          kind="ExternalInput")
    fc1_d = nc.dram_tensor("fc1", [128, CP * 2 * MLP], dt.float8e4,
                           kind="ExternalInput")
    fc2_d = nc.dram_tensor("fc2", [128, MP * 2 * C], dt.float8e4,
                           kind="ExternalInput")
    vecs_d = nc.dram_tensor("vecs", [128, CT * NV], dt.float32,
                            kind="ExternalInput")
    fb1_d = nc.dram_tensor("fb1", [128, MT], dt.float32, kind="ExternalInput")
    wcsum_d = nc.dram_tensor("wcsum", [1, C], dt.bfloat16, kind="ExternalInput")
    ident_d = nc.dram_tensor("ident", [128, 128], dt.float16,
                             kind="ExternalInput")
    out_d = nc.dram_tensor("out", [ROWS, C], dt.float32, kind="ExternalOutput")

    with tile.TileContext(nc) as tc, ExitStack() as ctx:
        const = ctx.enter_context(tc.tile_pool(name="const", bufs=1))
        inp = ctx.enter_context(tc.tile_pool(name="inp", bufs=18))
        xp = ctx.enter_context(tc.tile_pool(name="xp", bufs=26))
        sqp = ctx.enter_context(tc.tile_pool(name="sqp", bufs=7))
        h1p = ctx.enter_context(tc.tile_pool(name="h1p", bufs=13))
        h2p = ctx.enter_context(tc.tile_pool(name="h2p", bufs=8))
        a8p = ctx.enter_context(tc.tile_pool(name="a8p", bufs=25))
        bcp = ctx.enter_context(tc.tile_pool(name="bcp", bufs=6))
        tmpp = ctx.enter_context(tc.tile_pool(name="tmpp", bufs=8))
        rows = ctx.enter_context(tc.tile_pool(name="rows", bufs=6))
        rows1 = ctx.enter_context(tc.tile_pool(name="rows1", bufs=8))
        uap = ctx.enter_context(tc.tile_pool(name="uap", bufs=7))
        outp = ctx.enter_context(tc.tile_pool(name="outp", bufs=3))
        psum = ctx.enter_context(tc.tile_pool(name="psum", bufs=2, space="PSUM"))

        # ---- constants / weights; all input DMAs prefetched upfront ----
        ident_sb = const.tile([128, 128], dt.float16)
        nc.sync.dma_start(ident_sb[:], ident_d[:])
        vecs_sb = const.tile([128, CT * NV], dt.float32)
        nc.sync.dma_start(vecs_sb[:], vecs_d[:])
        fb1_sb = const.tile([128, MT], dt.float32)
        nc.sync.dma_start(fb1_sb[:], fb1_d[:])
        wcsum_sb = const.tile([1, C], dt.bfloat16)
        nc.sync.dma_start(wcsum_sb[:], wcsum_d[:])

        in_tiles = [dict() for _ in range(NG)]

        def dma_group_inputs(g):
            r0 = g * TG
            for s_, src_ in ((0, rgb_d), (1, dep_d)):
                for tt_ in range(NTT):
                    it_ = inp.tile([128, C], dt.float16, tag="in",
                                   name=f"in_{g}_{s_}_{tt_}")
                    nc.sync.dma_start(
                        it_[:], src_[r0 + tt_ * 128: r0 + (tt_ + 1) * 128, :])
                    in_tiles[g][s_, tt_] = it_

        dma_group_inputs(0)
        wc_sb = const.tile([128, CP, 2, C], dt.float8e4)
        nc.sync.dma_start(wc_sb[:, :, :, :], wc_d[:, :])
        dma_group_inputs(1)
        fc1_sb = const.tile([128, CP, 2, MLP], dt.float8e4)
        nc.sync.dma_start(fc1_sb[:, :, :, :], fc1_d[:, :])
        fc2_sb = const.tile([128, MP, 2, C], dt.float8e4)
        nc.sync.dma_start(fc2_sb[:, :, :, :], fc2_d[:, :])

        ones16 = const.tile([128, 1], dt.float16)
        nc.vector.memset(ones16[:], 1.0)
        sqrtc_f32 = const.tile([1, 128], dt.float32)
        nc.vector.memset(sqrtc_f32[:], float(np.sqrt(C)))
        sqrtc_row = const.tile([1, 128], dt.float32r)
        with nc.allow_low_precision("fp32r bcast lhsT"):
            nc.vector.tensor_copy(sqrtc_row[:], sqrtc_f32[:])
        isqrtc_row_b = const.tile([1, 128], dt.bfloat16)
        nc.vector.memset(isqrtc_row_b[:], float(1.0 / np.sqrt(C)))
        ceps_ap = const.tile([1, 1], dt.float32)
        nc.vector.memset(ceps_ap[:], float(C * EPS))

        def vec(idx, j):
            return vecs_sb[:, j * NV + idx: j * NV + idx + 1]

        x = [None] * NG            # (s, j) -> [128,TG] fp16 residual tiles
        st_rows = [None] * NG
        h1hi = [None] * NG         # (s, kp) -> [128,2,TG] fp8 pairs
        h1lo = [None] * NG
        h2pair = [None] * NG
        apairs = [None] * NG       # (s, mp) -> [128,2,TG] fp8 gelu pairs

        def stage_L(g):
            """PE transpose (inputs already DMA'd), DVE blend -> x0.

            Also prefetches the NEXT group's input DMAs (one stage-cycle
            of lead time) so transposes never chase the DMA engine."""
            if g >= 1 and g + 1 < NG:
                dma_group_inputs(g + 1)
            xg = {}
            for j in range(CT):
                pt = {}
                for s in (0, 1):
                    p = psum.tile([128, TG], dt.float16, tag="tp", bufs=3,
                                  padded_shape=[128, 1024],
                                  name=f"pt_{g}_{s}_{j}")
                    for tt_ in range(NTT):
                        nc.tensor.transpose(
                            p[:, tt_ * 128:(tt_ + 1) * 128],
                            in_tiles[g][s, tt_][:, j * 128:(j + 1) * 128],
                            ident_sb[:])
                    pt[s] = p
                t1 = tmpp.tile([128, TG], dt.float16, tag="bl",
                               name=f"t1_{g}_{j}")
                nc.vector.tensor_scalar(t1[:], pt[1][:], vec(V_A2, j),
                                        vec(V_A3, j), Alu.mult, Alu.add)
                x0r = xp.tile([128, TG], dt.float16, tag="res",
                              name=f"x0r_{g}_{j}")
                nc.vector.scalar_tensor_tensor(x0r[:], pt[0][:], vec(V_A1, j),
                                               t1[:], Alu.mult, Alu.add)
                t2 = tmpp.tile([128, TG], dt.float16, tag="bl",
                               name=f"t2_{g}_{j}")
                nc.vector.tensor_scalar(t2[:], pt[0][:], vec(V_D2, j),
                                        vec(V_D3, j), Alu.mult, Alu.add)
                x0d = xp.tile([128, TG], dt.float16, tag="res",
                              name=f"x0d_{g}_{j}")
                nc.vector.scalar_tensor_tensor(x0d[:], pt[1][:], vec(V_D1, j),
                                               t2[:], Alu.mult, Alu.add)
                xg[0, j] = x0r
                xg[1, j] = x0d
            x[g] = xg

        def ln_stats(g, name):
            """LN stats over channels for both streams of group g.

            Returns {('r'|'d'): (rrow_f32r, mr_bf16)}; rows are [1,TG].
            """
            out = {}
            for s in (0, 1):
                sfx = "r" if s == 0 else "d"
                sq = []
                for j in range(CT):
                    sqt = sqp.tile([128, TG], dt.float16, tag="sq",
                                   name=f"sq_{name}_{s}_{j}")
                    nc.gpsimd.tensor_tensor(sqt[:], x[g][s, j][:],
                                            x[g][s, j][:], Alu.mult)
                    sq.append(sqt)
                stat = psum.tile([128, TG], dt.float32, tag="tp", bufs=3,
                                 name=f"stat_{name}_{s}")
                for j in range(CT):
                    nc.tensor.matmul(stat[0:1, :], ones16[:], x[g][s, j][:],
                                     tile_position=(0, 0),
                                     start=(j == 0), stop=(j == CT - 1))
                    nc.tensor.matmul(stat[32:33, :], ones16[:], sq[j][:],
                                     tile_position=(0, 32),
                                     start=(j == 0), stop=(j == CT - 1))
                sq1 = rows.tile([1, TG], dt.float32, tag="rows",
                                name=f"sq1_{name}_{s}")
                nc.scalar.square(sq1[:], stat[0:1, :])
                u = rows.tile([1, TG], dt.float32, tag="rows",
                              name=f"u_{name}_{s}")
                nc.vector.scalar_tensor_tensor(u[:], sq1[:], -1.0 / C,
                                               stat[32:33, :], Alu.mult,
                                               Alu.add)
                std = rows.tile([1, TG], dt.float32, tag="rows",
                                name=f"std_{name}_{s}")
                nc.scalar.activation(std[:], u[:], Act.Sqrt,
                                     bias=ceps_ap[0:1, 0:1], scale=1.0)
                rrow = rows1.tile([1, TG], dt.float32r, tag="rows1", bufs=4,
                                  name=f"r_{name}_{s}")
                with nc.allow_low_precision("fp32r bcast rows"):
                    nc.vector.reciprocal(rrow[:], std[:])
                mr = rows1.tile([1, TG], dt.bfloat16, tag="rows1b", bufs=4,
                                name=f"mr_{name}_{s}")
                nc.vector.tensor_tensor(mr[:], stat[0:1, :], rrow[:], Alu.mult)
                out[sfx] = (rrow, mr)
            return out

        def bcast_r16(rrow, name):
            """rsqrt row broadcast: K=1 PE matmul + DVE copy to fp16 SBUF."""
            bc = psum.tile([128, TG], dt.float32, tag="tp", bufs=3,
                           name=f"bc_{name}")
            nc.tensor.matmul(bc[:], sqrtc_row[0:1, :], rrow[:],
                             start=True, stop=True)
            bc16 = bcp.tile([128, TG], dt.float16, tag="bc16",
                            name=f"bc16_{name}")
            nc.vector.tensor_copy(bc16[:], bc[:])
            return bc16

        def stage_W(g):
            """norm1 apply (fp8 hi/lo split) + Wc swap + residual -> x1."""
            st = st_rows[g]
            hhig, hlog = {}, {}
            for s in (0, 1):
                sfx = "r" if s == 0 else "d"
                bc16 = bcast_r16(st[sfx][0], f"n1_{g}_{s}")
                for kp in range(CP):
                    hip = h1p.tile([128, 2, TG], dt.float8e4, tag="h1",
                                   name=f"h1hi_{g}_{s}_{kp}")
                    lop = h1p.tile([128, 2, TG], dt.float8e4, tag="h1",
                                   name=f"h1lo_{g}_{s}_{kp}")
                    for i in (0, 1):
                        j = 2 * kp + i
                        t_ = tmpp.tile([128, TG], dt.float16, tag="bl",
                                       name=f"h1t_{g}_{s}_{j}")
                        nc.gpsimd.tensor_tensor(t_[:], x[g][s, j][:], bc16[:],
                                                Alu.mult)
                        nc.gpsimd.tensor_copy(hip[:, i, :], t_[:])
                        nc.gpsimd.tensor_tensor(lop[:, i, :], t_[:],
                                                hip[:, i, :], Alu.subtract)
                    hhig[s, kp] = hip
                    hlog[s, kp] = lop
            h1hi[g], h1lo[g] = hhig, hlog
            for s, o in ((0, 1), (1, 0)):
                mr = st["r" if s == 0 else "d"][1]
                # mo-halves of 2 so only 2 "acc" psum banks are held at once
                for half in range(3):
                    mos = (2 * half, 2 * half + 1)
                    accs = {}
                    for mo in mos:
                        accs[mo] = psum.tile([128, TG], dt.float32, tag="acc",
                                             bufs=3, name=f"g_{g}_{s}_{mo}")
                    first = True
                    for part in (hhig, hlog):
                        for kp in range(CP):
                            for mo in mos:
                                nc.tensor.matmul(
                                    accs[mo][:],
                                    wc_sb[:, kp, :, mo * 128:(mo + 1) * 128],
                                    part[s, kp][:, :, :],
                                    start=first and (kp == 0), stop=False,
                                    perf_mode=PerfMode.DoubleRow)
                        first = False
                    for mo in mos:
                        nc.tensor.matmul(
                            accs[mo][:],
                            wcsum_sb[0:1, mo * 128:(mo + 1) * 128],
                            mr[:], start=False, stop=True,
                            skip_group_check=True)
                        # x1_o = acc/WSCALE + x0_o (pb == 0), o = other stream
                        nc.vector.scalar_tensor_tensor(x[g][o, mo][:],
                                                       accs[mo][:],
                                                       vec(V_SCLW, mo),
                                                       x[g][o, mo][:],
                                                       Alu.mult, Alu.add)

        def stage_Mloop(g):
            """norm2 apply + interleaved-stream fc1/GELU/fc2(co=0) loop."""
            st = st_rows[g]
            h2g = {}
            for s in (0, 1):
                sfx = "r" if s == 0 else "d"
                rrow, mr = st[sfx]
                bc16 = bcast_r16(rrow, f"n2_{g}_{s}")
                bcm = psum.tile([128, TG], dt.float32, tag="tp", bufs=3,
                                name=f"bcm_{g}_{s}")
                nc.tensor.matmul(bcm[:], isqrtc_row_b[0:1, :], mr[:],
                                 start=True, stop=True)
                bcm16 = bcp.tile([128, TG], dt.float16, tag="bc16",
                                 name=f"bcm16_{g}_{s}")
                nc.vector.tensor_copy(bcm16[:], bcm[:])
                for kp in range(CP):
                    pair = h2p.tile([128, 2, TG], dt.float8e4, tag="h2",
                                    name=f"h2_{g}_{s}_{kp}")
                    for i in (0, 1):
                        j = 2 * kp + i
                        t_ = tmpp.tile([128, TG], dt.float16, tag="bl",
                                       name=f"h2t_{g}_{s}_{j}")
                        nc.gpsimd.tensor_tensor(t_[:], x[g][s, j][:], bc16[:],
                                                Alu.mult)
                        nc.gpsimd.tensor_tensor(pair[:, i, :], t_[:],
                                                bcm16[:], Alu.subtract)
                    h2g[s, kp] = pair
            # interleaved m-loop: ACT (gelu) and PE run concurrently; only
            # fc2 co=0 accumulates in-loop (psum pressure), rest in Mtail.
            acc0 = {}
            ap_g = {}
            apair_cur = {}
            for s in (0, 1):
                acc0[s] = psum.tile([128, TG], dt.float32, tag="acc", bufs=3,
                                    name=f"acc0_{g}_{s}")
            for m in range(MT):
                for s in (0, 1):
                    pf = psum.tile([128, TG], dt.float32, tag="ps", bufs=2,
                                   name=f"pf_{g}_{s}_{m}")
                    for kp in range(CP):
                        nc.tensor.matmul(
                            pf[:],
                            fc1_sb[:, kp, :, m * 128:(m + 1) * 128],
                            h2g[s, kp][:, :, :],
                            start=(kp == 0), stop=(kp == CP - 1),
                            perf_mode=PerfMode.DoubleRow)
                    if m % 2 == 0:
                        apair_cur[s] = a8p.tile([128, 2, TG], dt.float8e4,
                                                tag="a8",
                                                name=f"a_{g}_{s}_{m // 2}")
                        ap_g[s, m // 2] = apair_cur[s]
                    nc.scalar.activation(apair_cur[s][:, m % 2, :], pf[:],
                                         Act.Gelu, bias=fb1_sb[:, m:m + 1],
                                         scale=float(1.0 / WSCALE))
                    if m % 2 == 1:
                        mp = m // 2
                        nc.tensor.matmul(
                            acc0[s][:],
                            fc2_sb[:, mp, :, 0:128],
                            apair_cur[s][:, :, :],
                            start=(mp == 0), stop=(mp == MP - 1),
                            perf_mode=PerfMode.DoubleRow)
            apairs[g] = ap_g
            h2pair[g] = h2g
            for s in (0, 1):
                nc.vector.scalar_tensor_tensor(x[g][s, 0][:], acc0[s][:],
                                               vec(V_SCL, 0), x[g][s, 0][:],
                                               Alu.mult, Alu.add)

        def stage_Mtail(g):
            """fc2 co=1..5 swept densely from the persistent a8 pairs."""
            ap_g = apairs[g]
            for s in (0, 1):
                for chunk in ((1, 2), (3, 4), (5,)):
                    accs = {}
                    for co in chunk:
                        accs[co] = psum.tile([128, TG], dt.float32, tag="acc",
                                             bufs=3, name=f"acc_{g}_{s}_{co}")
                    for mp in range(MP):
                        for co in chunk:
                            nc.tensor.matmul(
                                accs[co][:],
                                fc2_sb[:, mp, :, co * 128:(co + 1) * 128],
                                ap_g[s, mp][:, :, :],
                                start=(mp == 0), stop=(mp == MP - 1),
                                perf_mode=PerfMode.DoubleRow)
                    for co in chunk:
                        nc.vector.scalar_tensor_tensor(x[g][s, co][:],
                                                       accs[co][:],
                                                       vec(V_SCL, co),
                                                       x[g][s, co][:],
                                                       Alu.mult, Alu.add)

        def stage_F(g):
            """final norm + modality mean + transpose out + DMA."""
            st = st_rows[g]
            bc_rr16 = bcast_r16(st["r"][0], f"nf_{g}_r")
            bc_rd16 = bcast_r16(st["d"][0], f"nf_{g}_d")
            bc_mrs = psum.tile([128, TG], dt.float32, tag="tp", bufs=3,
                               name=f"bcmrs_{g}")
            nc.tensor.matmul(bc_mrs[:], isqrtc_row_b[0:1, :], st["r"][1][:],
                             start=True, stop=False)
            nc.tensor.matmul(bc_mrs[:], isqrtc_row_b[0:1, :], st["d"][1][:],
                             start=False, stop=True)
            bc_mrs16 = bcp.tile([128, TG], dt.float16, tag="bc16",
                                name=f"bcmrs16_{g}")
            nc.vector.tensor_copy(bc_mrs16[:], bc_mrs[:])
            uas = []
            for j in range(CT):
                s1 = tmpp.tile([128, TG], dt.float16, tag="bl",
                               name=f"nf1_{g}_{j}")
                nc.vector.tensor_tensor(s1[:], x[g][0, j][:], bc_rr16[:],
                                        Alu.mult)
                s2 = tmpp.tile([128, TG], dt.float16, tag="bl",
                               name=f"nf2_{g}_{j}")
                nc.vector.tensor_tensor(s2[:], x[g][1, j][:], bc_rd16[:],
                                        Alu.mult)
                nc.gpsimd.tensor_tensor(s1[:], s1[:], s2[:], Alu.add)
                nc.gpsimd.tensor_tensor(s1[:], s1[:], bc_mrs16[:],
                                        Alu.subtract)
                ua = uap.tile([128, TG], dt.float16, tag="uaff",
                              name=f"ua_{g}_{j}")
                nc.vector.tensor_scalar(ua[:], s1[:], vec(V_WFH, j),
                                        vec(V_BF, j), Alu.mult, Alu.add)
                uas.append(ua)
            r0 = g * TG
            for tt_ in range(NTT):
                po = psum.tile([128, TG], dt.float16, tag="tp", bufs=3,
                               padded_shape=[128, 1024], name=f"po_{g}_{tt_}")
                po2 = psum.tile([128, TG], dt.float16, tag="tp", bufs=3,
                                padded_shape=[128, 1024],
                                name=f"po2_{g}_{tt_}")
                for j in range(CT):
                    dst = (po[:, j * 128:(j + 1) * 128] if j < 4
                           else po2[:, (j - 4) * 128:(j - 3) * 128])
                    nc.tensor.transpose(
                        dst, uas[j][:, tt_ * 128:(tt_ + 1) * 128], ident_sb[:])
                ot = outp.tile([128, C], dt.float32, tag="ot",
                               name=f"ot_{g}_{tt_}")
                nc.scalar.copy(ot[:, 0:512], po[:, :])
                nc.scalar.copy(ot[:, 512:768], po2[:, 0:256])
                nc.sync.dma_start(
                    out_d[r0 + tt_ * 128: r0 + (tt_ + 1) * 128, :], ot[:])

        def S1(g):
            st_rows[g] = ln_stats(g, f"n1_{g}")

        def S2(g):
            st_rows[g] = ln_stats(g, f"n2_{g}")

        def SF(g):
            st_rows[g] = ln_stats(g, f"nf_{g}")

        # software pipeline: next group's load/stats fill this group's
        # PE dependency gaps (esp. around the MLP and Wc phases).
        sched = [(stage_L, 0), (S1, 0), (stage_W, 0), (S2, 0)]
        for g in range(NG):
            if g + 1 < NG:
                sched += [(stage_Mloop, g), (stage_Mtail, g),
                          (stage_L, g + 1), (S1, g + 1), (stage_W, g + 1),
                          (SF, g), (S2, g + 1), (stage_F, g)]
            else:
                sched += [(stage_Mloop, g), (stage_Mtail, g),
                          (SF, g), (stage_F, g)]
        for fn, g in sched:
            fn(g)

    if legalize:
        _legalize_waits(nc)
    nc.finalize()
    return nc


def _legalize_waits(nc):
    """Walrus ISA structs have at most 1-2 sync-wait slots per instruction,
    but Tile's wait assignment can emit more. Move excess waits onto
    same-engine NoOps inserted immediately before the offending
    instruction."""
    import bass_rust
    nop_i = [0]
    for f in nc.m.functions:
        for b in f.blocks:
            insts = b.instructions
            out = []
            changed = False
            for ins in insts:
                si = getattr(ins, "sync_info", None)
                waits = list(si.on_wait) if (si and si.on_wait) else []
                if len(waits) > 1:
                    eng = ins.engine
                    for w in waits[:-1]:
                        n = bass_rust.InstNoOp(name=f"I-nopw-{nop_i[0]}")
                        nop_i[0] += 1
                        n.engine = eng
                        n.sync_info = bass_rust.SyncInfo(
                            on_wait=[w], on_update=[])
                        out.append(n)
                    ins.sync_info = bass_rust.SyncInfo(
                        on_wait=[waits[-1]], on_update=list(si.on_update or []))
                    changed = True
                out.append(ins)
            if changed:
                b.instructions = out


def _prepare(inputs):
    """Host-side folding: per-channel vectors + fused/packed weights."""
    f = lambda k: np.asarray(inputs[k], np.float64)
    alpha = f("alpha").reshape(C)

    s_r = f("bn_rgb_w") / np.sqrt(f("bn_rgb_var") + EPS)
    t_r = f("bn_rgb_b") - f("bn_rgb_mean") * s_r
    s_d = f("bn_depth_w") / np.sqrt(f("bn_depth_var") + EPS)
    t_d = f("bn_depth_b") - f("bn_depth_mean") * s_d

    w_r = np.asarray(inputs["bn_rgb_w"], np.float32)
    w_d = np.asarray(inputs["bn_depth_w"], np.float32)
    idx_r = np.argsort(np.abs(w_r), kind="stable")[:K_EX]
    idx_d = np.argsort(np.abs(w_d), kind="stable")[:K_EX]
    mask_r = np.zeros(C, bool)
    mask_r[idx_r] = True
    mask_d = np.zeros(C, bool)
    mask_d[idx_d] = True

    A1 = np.where(mask_r, alpha * s_r, s_r)
    A2 = np.where(mask_r, (1 - alpha) * s_d, 0.0)
    A3 = np.where(mask_r, alpha * t_r + (1 - alpha) * t_d, t_r)
    D1 = np.where(mask_d, alpha * s_d, s_d)
    D2 = np.where(mask_d, (1 - alpha) * s_r, 0.0)
    D3 = np.where(mask_d, alpha * t_d + (1 - alpha) * t_r, t_d)

    qkv_w = f("qkv_w")
    Wv = qkv_w[2 * C:, :]
    Wc = f("proj_w") @ Wv
    w1, b1 = f("norm1_w"), f("norm1_b")
    Wc_f = Wc * w1[None, :]
    pb = f("proj_b") + Wc @ b1
    assert np.allclose(pb, 0.0), "kernel folds pb==0 into the Wc descale slot"
    wc_rowsum = Wc_f.sum(axis=1)

    w2, b2 = f("norm2_w"), f("norm2_b")
    fc1_f = f("fc1_w") * w2[None, :]
    fb1 = f("fc1_b") + f("fc1_w") @ b2
    fc2_w = f("fc2_w")
    fc2_b = f("fc2_b")
    assert np.allclose(fc2_b, 0.0), "kernel folds fc2_b==0 into V_SCL slot"
    wfh = 0.5 * f("normf_w")

    bf16 = ml_dtypes.bfloat16
    fp8 = ml_dtypes.float8_e4m3

    def pack_lhsT_pairs(wT, kp, m):
        # wT: [kp*256, m] -> [128, kp*2*m], [p, ((q*2+i)*m)+col] =
        #   wT[(2q+i)*128+p, col]   (DoubleRow k-pair layout)
        return np.ascontiguousarray(
            wT.reshape(kp, 2, 128, m).transpose(2, 0, 1, 3).reshape(
                128, kp * 2 * m))

    wc_pack = pack_lhsT_pairs(
        np.ascontiguousarray(Wc_f.T) * WSCALE, CP, C).astype(fp8)
    fc1_pack = pack_lhsT_pairs(
        np.ascontiguousarray(fc1_f.T) * WSCALE, CP, MLP).astype(fp8)
    fc2_pack = pack_lhsT_pairs(
        np.ascontiguousarray(fc2_w.T) * WSCALE, MP, C).astype(fp8)

    scl = np.full(C, 1.0 / WSCALE)
    vv = [A1, A2, A3, D1, D2, D3, scl, scl, wfh, f("normf_b")]
    vecs = np.stack(vv, axis=-1).astype(np.float32)          # [C, NV]
    vecs = vecs.reshape(CT, 128, NV).transpose(1, 0, 2).reshape(128, CT * NV)
    vecs = np.ascontiguousarray(vecs)
    fb1_pack = np.ascontiguousarray(
        fb1.astype(np.float32).reshape(MT, 128).T)           # [128, MT]

    return {
        "wc": wc_pack,
        "fc1": fc1_pack,
        "fc2": fc2_pack,
        "vecs": vecs,
        "fb1": fb1_pack,
        "wcsum": (-wc_rowsum * WSCALE / np.sqrt(C)).astype(bf16).reshape(1, C),
        "ident": np.eye(128, dtype=np.float16),
    }


def _get_runner():
    """Build the Bass module once and cache a jitted shard_map executor."""
    if "runner" in _CACHE:
        return _CACHE["runner"]
    import jax
    from jax.sharding import Mesh, PartitionSpec
    from jax.experimental.shard_map import shard_map
    from concourse import bass2jax

    nc = _build_nc()
    bass2jax.install_neuronx_cc_hook()
    partition_name = (nc.partition_id_tensor.name
                      if nc.partition_id_tensor else None)
    in_names, out_names, out_avals = [], [], []
    for alloc in nc.m.functions[0].allocations:
        if not isinstance(alloc, mybir.MemoryLocationSet):
            continue
        name = alloc.memorylocations[0].name
        if alloc.kind == "ExternalInput":
            if name != partition_name:
                in_names.append(name)
        elif alloc.kind == "ExternalOutput":
            out_names.append(name)
            out_avals.append(jax.core.ShapedArray(
                tuple(alloc.tensor_shape), mybir.dt.np(alloc.dtype)))
    all_in_names = list(in_names) + list(out_names)
    if partition_name is not None:
        all_in_names.append(partition_name)

    def _body(*args):
        operands = list(args)
        if partition_name is not None:
            operands.append(bass2jax.partition_id_tensor())
        return tuple(bass2jax._bass_exec_p.bind(
            *operands, out_avals=tuple(out_avals),
            in_names=tuple(all_in_names), out_names=tuple(out_names),
            lowering_input_output_aliases=(),
            sim_require_finite=True, sim_require_nnan=True, nc=nc))

    devices = jax.devices()[:N_CORES]
    mesh = Mesh(np.asarray(devices), ("core",))
    sharded_args = {"rgb", "dep"}
    in_specs = tuple(
        PartitionSpec("core") if n in sharded_args else PartitionSpec()
        for n in in_names) + (PartitionSpec("core"),) * len(out_names)
    fn = jax.jit(
        shard_map(_body, mesh=mesh,
                  in_specs=in_specs,
                  out_specs=(PartitionSpec("core"),) * len(out_names),
                  check_rep=False),
        keep_unused=True)
    zeros = [jax.device_put(
        np.zeros((a.shape[0] * N_CORES,) + tuple(a.shape[1:]), a.dtype))
        for a in out_avals]
    _CACHE["runner"] = (fn, in_names, zeros, jax)
    return _CACHE["runner"]


def kernel(**inputs) -> np.ndarray:
    rgb = np.asarray(inputs["rgb"], np.float32).astype(np.float16)
    dep = np.asarray(inputs["depth"], np.float32).astype(np.float16)
    consts = _prepare(inputs)

    fn, in_names, zeros, jax = _get_runner()
    vals = {
        "rgb": np.ascontiguousarray(rgb.reshape(ROWS * N_CORES, C)),
        "dep": np.ascontiguousarray(dep.reshape(ROWS * N_CORES, C)),
    }
    vals.update(consts)
    args = [vals[n] for n in in_names] + list(zeros)
    outs = fn(*args)
    out = np.asarray(outs[0]).reshape(B, T, C)
    return out


if __name__ == "__main__":
    print("built module ok" if _build_nc() else "")
